# revision 49
# baseline (speedup 1.0000x reference)
"""FCOS head (nn_FCOS_73787538145418) Trainium2 Bass kernel.

Sharding: data-parallel, one image per NeuronCore (B=8 across 8 cores),
weights replicated. Each core runs the identical SPMD NEFF over its image.

Algorithm: 1D Winograd F(4,3) along W (direct 3-tap conv along H) for the
p3/p4 levels, direct conv for p5. Halves tensor-engine rows for stems and
prediction convs on p3/p4. fp16 operands (1 cyc/row on the PE), fp32 PSUM.
Per conv layer: W-transformed input Xt[j=0..5] (host-computed for the
feature, DVE-computed between layers), 6 PSUM groups m_j accumulated over
(k=3 H-taps x 2 ci chunks), inverse transform y = AT m on DVE/gpsimd,
bias+ReLU on the scalar engine writing a tiled-column spatial layout
[rows, 4, W/4+1] that keeps the next in-transform's reads contiguous.
Output is [25, 5376] channel-major per core; host transposes/stacks.
"""
import sys

if '/opt/trn_rl_repo' not in sys.path:
    sys.path.insert(0, '/opt/trn_rl_repo')

import numpy as np

import concourse.mybir as mybir
from concourse import bacc
import concourse.tile as tile
from concourse.bass_utils import run_bass_kernel_spmd

P = 128
NCH = 2                 # 256 channels = 2 chunks of 128
NL = 4                  # stem depth
NPIX_TOTAL = 5376
F16 = mybir.dt.float16
F32 = mybir.dt.float32
AL = mybir.AluOpType
AF = mybir.ActivationFunctionType

# F(4,3) Winograd (points [0, 1, -1, 2, -2])
G_MAT = np.array([
    [1 / 4, 0, 0], [-1 / 6, -1 / 6, -1 / 6], [-1 / 6, 1 / 6, -1 / 6],
    [1 / 24, 1 / 12, 1 / 6], [1 / 24, -1 / 12, 1 / 6], [0, 0, 1]])

_cached = {}
_run_opts = {}   # extra kwargs for run_bass_kernel_spmd (test harness: trace)
_last = {}       # last BassKernelResults (test harness reads exec_time_ns)


# ---------------------------------------------------------------- views
def _xt_view(flat, off, H, T):
    n = NCH * 6 * (H + 2) * T
    return flat[:, off:off + n].rearrange(
        "p (c j r t) -> p c j r t", c=NCH, j=6, r=H + 2)


def _y_view(flat, off, H, T):
    # tiled-column spatial layout: col = 4*tw + f, tw in [0, T], f in [0, 4)
    n = NCH * (H + 2) * 4 * (T + 1)
    return flat[:, off:off + n].rearrange(
        "p (c r f t) -> p c r f t", c=NCH, r=H + 2, f=4)


def _pad_view(flat, off, H, W):
    n = NCH * (H + 2) * (W + 2)
    return flat[:, off:off + n].rearrange(
        "p (c h w) -> p c h w", c=NCH, h=H + 2, w=W + 2)


def _zero_y_ring(nc, y, H, T):
    nc.vector.memset(y[:, :, 0], 0.0)
    nc.vector.memset(y[:, :, H + 1], 0.0)
    nc.vector.memset(y[:, :, 1:H + 1, 0, 0], 0.0)
    nc.vector.memset(y[:, :, 1:H + 1, 1:4, T], 0.0)


def _zero_ring(nc, v, H, W):
    for c in range(NCH):
        nc.vector.memset(v[:, c, 0, :], 0.0)
        nc.vector.memset(v[:, c, H + 1, :], 0.0)
        nc.vector.memset(v[:, c, 1:H + 1, 0], 0.0)
        nc.vector.memset(v[:, c, 1:H + 1, W + 1], 0.0)


# ------------------------------------------------------- winograd pieces
def _intrans(nc, it_pool, y, xt, H, T, tag, r0=1, r1=None):
    """W-direction F(4,3) input transform: y spatial -> xt[j], rows r0..r1.

    Callers split the row range so the first band's matmuls can start
    after the first chunk instead of the full-image transform."""
    if r1 is None:
        r1 = H + 1
    H = r1 - r0
    xa = [y[:, :, r0:r1, a, 0:T] for a in range(4)]
    xa.append(y[:, :, r0:r1, 0, 1:T + 1])
    xa.append(y[:, :, r0:r1, 1, 1:T + 1])

    def scr(nm):
        return it_pool.tile([P, NCH, H, T], F16, tag="it",
                            name=f"it_{tag}_{nm}")[:]
    V, GP = nc.vector, nc.gpsimd
    g = scr("g"); V.tensor_tensor(g, xa[3], xa[1], AL.subtract)
    h = scr("h"); V.tensor_tensor(h, xa[4], xa[2], AL.subtract)
    V.scalar_tensor_tensor(xt[:, :, 3, r0:r1], g, 2.0, h, AL.mult, AL.add)
    V.scalar_tensor_tensor(xt[:, :, 4, r0:r1], g, -2.0, h, AL.mult, AL.add)
    m = scr("m"); V.tensor_tensor(m, xa[5], xa[3], AL.subtract)
    V.scalar_tensor_tensor(xt[:, :, 5, r0:r1], g, -4.0, m, AL.mult, AL.add)
    f = scr("f"); V.tensor_tensor(f, xa[0], xa[2], AL.subtract)
    V.scalar_tensor_tensor(xt[:, :, 0, r0:r1], f, 4.0, h, AL.mult, AL.add)
    u = scr("u"); V.tensor_tensor(u, xa[1], xa[2], AL.subtract)
    v = scr("v"); V.tensor_tensor(v, xa[4], xa[3], AL.subtract)
    V.scalar_tensor_tensor(xt[:, :, 2, r0:r1], u, 4.0, v, AL.mult, AL.add)
    p_ = scr("p"); V.tensor_tensor(p_, xa[1], xa[2], AL.add)
    q = scr("q"); GP.tensor_tensor(q, xa[3], xa[4], AL.add)
    V.scalar_tensor_tensor(xt[:, :, 1, r0:r1], p_, -4.0, q, AL.mult, AL.add)


def _wino_unit(nc, psum_pool, ot_pool, ya_pool, lhsT_fn, xt, r0, R, T,
               nlo, nhi, tag):
    """One band: 6 PSUM groups (3k x 2ci matmuls each) + inverse transform.

    lhsT_fn(c, j, k) -> weight AP [K=128, M]; output written to partitions
    nlo:nhi of psum/scr tiles. Returns yact tile view [nlo:nhi, R, 4, T]."""
    def scr(nm):
        t = ot_pool.tile([P, R, T], F16, tag="ot", name=f"ot_{tag}_{nm}")
        return t[nlo:nhi]

    cs = []
    def mm(j):
        # 6 accumulating matmuls into one PSUM group, then a scalar-engine
        # copy to fp16 SBUF (PSUM allows only one engine-instruction input;
        # the copy also releases the PSUM bank early)
        ps = psum_pool.tile([P, R, T], F32, tag="ps", name=f"ps_{tag}_{j}")
        kk = 0
        for c in range(NCH):
            for k in range(3):
                nc.tensor.matmul(ps[nlo:nhi], lhsT_fn(c, j, k),
                                 xt[:, c, j, r0 + k:r0 + k + R, :],
                                 start=(kk == 0), stop=(kk == 5))
                kk += 1
        cj = scr(f"c{j}")
        nc.scalar.activation(cj, ps[nlo:nhi], AF.Copy)
        cs.append(cj)

    ya = ya_pool.tile([P, R, 4, T], F16, tag="ya", name=f"ya_{tag}")
    V, GP = nc.vector, nc.gpsimd

    mm(0); mm(1); mm(2)
    s = scr("s"); V.tensor_tensor(s, cs[1], cs[2], AL.add)
    d = scr("d"); V.tensor_tensor(d, cs[1], cs[2], AL.subtract)
    mm(3); mm(4)
    # DVE TensorTensor runs in 2x_1p mode (~4x cheaper than gpsimd);
    # gpsimd keeps only S/D so y1/y2 can start while DVE finishes u/y0
    u = scr("u"); V.tensor_tensor(u, cs[0], s, AL.add)
    S = scr("S"); GP.tensor_tensor(S, cs[3], cs[4], AL.add)
    D = scr("D"); GP.tensor_tensor(D, cs[3], cs[4], AL.subtract)
    V.scalar_tensor_tensor(ya[nlo:nhi, :, 1, :], D, 2.0, d, AL.mult, AL.add)
    V.scalar_tensor_tensor(ya[nlo:nhi, :, 2, :], S, 4.0, s, AL.mult, AL.add)
    mm(5)
    V.tensor_tensor(ya[nlo:nhi, :, 0, :], u, S, AL.add)
    v3 = scr("v3")
    V.scalar_tensor_tensor(v3, D, 8.0, d, AL.mult, AL.add)
    V.tensor_tensor(ya[nlo:nhi, :, 3, :], v3, cs[5], AL.add)
    return ya


def _wino_layer(nc, pools, wt, xt, ydst, bias_ap, H, T, tag, plain=False):
    """Full 256->256 W-winograd conv + bias + relu.

    plain=False: ydst is the tiled-column layout (feeds next in-transform).
    plain=True: ydst is a plain padded [c, H+2, W+2] view (feeds direct
    prediction convs); the activation collapses to one instruction."""
    psum_pool, ot_pool, ya_pool = pools
    bands = [(0, 32), (32, 32)] if H == 64 else [(0, H)]
    W = 4 * T
    for bi, (r0, R) in enumerate(bands):
        for o in range(NCH):
            ya = _wino_unit(nc, psum_pool, ot_pool, ya_pool,
                            lambda c, j, k: wt[:, c, o, j, k, :],
                            xt, r0, R, T, 0, P, f"{tag}{bi}{o}")
            rows = slice(r0 + 1, r0 + 1 + R)
            if plain:
                dv = ydst[:, o, rows, 1:W + 1].rearrange(
                    "p r (t f) -> p r f t", f=4)
                nc.scalar.activation(dv, ya[:], AF.Relu, bias=bias_ap[:, o])
            else:
                nc.scalar.activation(ydst[:, o, rows, 1:4, 0:T],
                                     ya[:, :, 0:3, :], AF.Relu,
                                     bias=bias_ap[:, o])
                nc.scalar.activation(ydst[:, o, rows, 0, 1:T + 1],
                                     ya[:, :, 3, :], AF.Relu,
                                     bias=bias_ap[:, o])


def _wino_preds(nc, pools, stage_pool, pwct, pwbt, pbc_t, pbb_t,
                xtc, xtb, out_d, H, T, pix_base, tag):
    """cls(20ch) + box/ctr(5ch) W-winograd pred convs + bias (no relu)."""
    psum_pool, ot_pool, ya_pool = pools
    bands = [(0, 32), (32, 32)] if H == 64 else [(0, H)]
    for bi, (r0, R) in enumerate(bands):
        n = R * T * 4
        c0 = pix_base + r0 * T * 4
        for hi, (pw, nout, xt, pb, olo) in enumerate(
                ((pwct, 20, xtc, pbc_t, 0), (pwbt, 5, xtb, pbb_t, 20))):
            ya = _wino_unit(nc, psum_pool, ot_pool, ya_pool,
                            lambda c, j, k: pw[:, c, j, k, :],
                            xt, r0, R, T, 0, nout, f"{tag}p{bi}{hi}")
            st = stage_pool.tile([32, R, T, 4], F32, tag="st",
                                 name=f"st_{tag}{bi}{hi}")
            stv = st.rearrange("p r t f -> p r f t")
            nc.scalar.activation(stv[0:nout], ya[0:nout], AF.Identity,
                                 bias=pb[0:nout])
            nc.sync.dma_start(
                out_d[olo:olo + nout, c0:c0 + n],
                st[0:nout].rearrange("p r t f -> p (r t f)"))


# ------------------------------------------------------- p5 direct path
def _conv_layer(nc, psum_pool, wt, src, dst, bias_ap, H, W, R, tag):
    n_tiles = H // R
    for o in range(NCH):
        pss = [psum_pool.tile([P, R, W], F32, tag="ps",
                              name=f"ps_{tag}_{o}_{it}")
               for it in range(n_tiles)]
        k = 0
        for c in range(NCH):
            for ky in range(3):
                for kx in range(3):
                    lhsT = wt[:, c, o, ky * 3 + kx, :]
                    for it in range(n_tiles):
                        r0 = it * R
                        rhs = src[:, c, r0 + ky:r0 + ky + R, kx:kx + W]
                        nc.tensor.matmul(pss[it][:], lhsT, rhs,
                                         start=(k == 0), stop=(k == 17))
                    k += 1
        for it in range(n_tiles):
            r0 = it * R
            nc.scalar.activation(dst[:, o, r0 + 1:r0 + 1 + R, 1:W + 1],
                                 pss[it][:], AF.Relu, bias=bias_ap[:, o])


def _preds5_head(nc, psum_pool, stage_pool, pw, pb_t, tower, out_d,
                 H, W, R, pix_base, olo, nout, tag, it):
    """One R-row band of ONE direct prediction head (cls or box/ctr)."""
    r0 = it * R
    ps = psum_pool.tile([P, R, W], F32, tag="ps", name=f"ph_{tag}_{it}")
    k = 0
    for c in range(NCH):
        for ky in range(3):
            for kx in range(3):
                t = ky * 3 + kx
                nc.tensor.matmul(ps[0:nout], pw[:, c, t, :],
                                 tower[:, c, r0 + ky:r0 + ky + R, kx:kx + W],
                                 start=(k == 0), stop=(k == 17))
                k += 1
    st = stage_pool.tile([32, R, W, 1], F32, tag="st", name=f"sh_{tag}_{it}")
    sf = st.rearrange("p r w o -> p (r w o)")
    nc.scalar.activation(sf[0:nout], ps[0:nout].rearrange("p r w -> p (r w)"),
                         AF.Identity, bias=pb_t[0:nout])
    c0 = pix_base + r0 * W
    nc.sync.dma_start(out_d[olo:olo + nout, c0:c0 + R * W], sf[0:nout])


def _preds5(nc, psum_pool, stage_pool, pwc, pwb, pbc_t, pbb_t,
            cls_tower, box_tower, out_d, H, W, R, pix_base, tag):
    n_tiles = H // R
    for it in range(n_tiles):
        r0 = it * R
        ps1 = psum_pool.tile([P, R, W], F32, tag="ps", name=f"pc_{tag}_{it}")
        ps2 = psum_pool.tile([P, R, W], F32, tag="ps", name=f"pb_{tag}_{it}")
        k = 0
        for c in range(NCH):
            for ky in range(3):
                for kx in range(3):
                    t = ky * 3 + kx
                    rc = cls_tower[:, c, r0 + ky:r0 + ky + R, kx:kx + W]
                    rb = box_tower[:, c, r0 + ky:r0 + ky + R, kx:kx + W]
                    nc.tensor.matmul(ps1[0:20], pwc[:, c, t, :], rc,
                                     start=(k == 0), stop=(k == 17))
                    nc.tensor.matmul(ps2[0:5], pwb[:, c, t, :], rb,
                                     start=(k == 0), stop=(k == 17))
                    k += 1
        st = stage_pool.tile([32, R, W, 1], F32, tag="st", name=f"s5_{tag}_{it}")
        sf = st.rearrange("p r w o -> p (r w o)")
        nc.scalar.activation(sf[0:20], ps1[0:20].rearrange("p r w -> p (r w)"),
                             AF.Identity, bias=pbc_t[0:20])
        st2 = stage_pool.tile([32, R, W, 1], F32, tag="st", name=f"s6_{tag}_{it}")
        sf2 = st2.rearrange("p r w o -> p (r w o)")
        nc.scalar.activation(sf2[0:5], ps2[0:5].rearrange("p r w -> p (r w)"),
                             AF.Identity, bias=pbb_t[0:5])
        c0 = pix_base + r0 * W
        nc.sync.dma_start(out_d[0:20, c0:c0 + R * W], sf[0:20])
        nc.sync.dma_start(out_d[20:25, c0:c0 + R * W], sf2[0:5])


# ------------------------------------------------------------ weight DMA
def _load_ww(nc, wtw_pool, sww_d, s, l, tag, fine=False):
    wt = wtw_pool.tile([P, NCH, NCH, 6, 3, P], F16, tag="ww",
                       name=f"ww_{tag}_{s}_{l}")
    if fine:
        # per-(o, j, c) splits so the first matmuls' deps clear quickly
        for o in range(NCH):
            for j in range(6):
                for c in range(NCH):
                    nc.sync.dma_start(wt[:, c, o, j], sww_d[s, l, :, c, o, j])
    else:
        for c in range(NCH):
            for o in range(NCH):
                nc.sync.dma_start(wt[:, c, o], sww_d[s, l, :, c, o])
    return wt


def _load_wp(nc, wtp_pool, swp_d, s, l, tag):
    wt = wtp_pool.tile([P, NCH, NCH, 9, P], F16, tag="wp",
                       name=f"wp_{tag}_{s}_{l}")
    for c in range(NCH):
        for o in range(NCH):
            nc.sync.dma_start(wt[:, c, o], swp_d[s, l, :, c, o])
    return wt


# ------------------------------------------------------------------ build
def _build():
    nc = bacc.Bacc("TRN2", target_bir_lowering=False, debug=False,
                   num_devices=8)

    # p3 Xt0 (host-transformed), shipped once per chain buffer
    xt3_d = nc.dram_tensor("xt3", (NCH, P, 6, 66, 16), F16,
                           kind="ExternalInput")
    xt4_d = nc.dram_tensor("xt4", (NCH, P, 6, 34, 8), F16,
                           kind="ExternalInput")
    x5_d = nc.dram_tensor("x5", (NCH, P, 18, 18), F16, kind="ExternalInput")
    sww_d = nc.dram_tensor("sww", (2, NL, P, NCH, NCH, 6, 3, P), F16,
                           kind="ExternalInput")
    swp_d = nc.dram_tensor("swp", (2, NL, P, NCH, NCH, 9, P), F16,
                           kind="ExternalInput")
    sb_d = nc.dram_tensor("sb", (2, NL, NCH, P, 1), F32, kind="ExternalInput")
    pwct_d = nc.dram_tensor("pwct", (P, NCH, 6, 3, 20), F16,
                            kind="ExternalInput")
    pwbt_d = nc.dram_tensor("pwbt", (P, NCH, 6, 3, 5), F16,
                            kind="ExternalInput")
    pwc5_d = nc.dram_tensor("pwc5", (P, NCH, 9, 20), F16,
                            kind="ExternalInput")
    pwb5_d = nc.dram_tensor("pwb5", (P, NCH, 9, 5), F16,
                            kind="ExternalInput")
    pbc_d = nc.dram_tensor("pbc", (20, 1), F32, kind="ExternalInput")
    pbb_d = nc.dram_tensor("pbb", (5, 1), F32, kind="ExternalInput")
    out_d = nc.dram_tensor("out", (25, NPIX_TOTAL), F32, kind="ExternalOutput")

    NXT3 = NCH * 6 * 66 * 16      # 12672
    NY3 = NCH * 66 * 68           # 8976
    NXT4 = NCH * 6 * 34 * 8       # 3264
    NY4 = NCH * 34 * 36           # 2448
    N5 = NCH * 18 * 18            # 648

    with tile.TileContext(nc) as tc:
        with (
            tc.tile_pool(name="resident", bufs=1) as res_pool,
            tc.tile_pool(name="wtw", bufs=2) as wtw_pool,
            tc.tile_pool(name="wtp", bufs=1) as wtp_pool,
            tc.tile_pool(name="psum", bufs=8, space="PSUM") as psum_pool,
            tc.tile_pool(name="stage", bufs=4) as stage_pool,
            tc.tile_pool(name="it", bufs=5) as it_pool,
            tc.tile_pool(name="ot", bufs=22) as ot_pool,
            tc.tile_pool(name="ya", bufs=5) as ya_pool,
        ):
            xtc_f = res_pool.tile([P, NXT3], F16, name="xtc")
            xtb_f = res_pool.tile([P, NXT3], F16, name="xtb")
            yc_f = res_pool.tile([P, NY3], F16, name="yc")
            yb_f = res_pool.tile([P, NY3], F16, name="yb")

            sbias = res_pool.tile([P, 2, NL, NCH, 1], F32, name="sbias")
            pwct = res_pool.tile([P, NCH, 6, 3, 20], F16, name="pwct")
            pwbt = res_pool.tile([P, NCH, 6, 3, 5], F16, name="pwbt")
            pwc5 = res_pool.tile([P, NCH, 9, 20], F16, name="pwc5")
            pwb5 = res_pool.tile([P, NCH, 9, 5], F16, name="pwb5")
            pbc_t = res_pool.tile([32, 1], F32, name="pbct")
            pbb_t = res_pool.tile([32, 1], F32, name="pbbt")

            xtc = _xt_view(xtc_f, 0, 64, 16)
            xtb = _xt_view(xtb_f, 0, 64, 16)
            yc = _y_view(yc_f, 0, 64, 16)
            yb = _y_view(yb_f, 0, 64, 16)
            # pass-b carvings
            xtc4 = _xt_view(yc_f, 0, 32, 8)
            yc4 = _y_view(yc_f, NXT4, 32, 8)
            xtb4 = _xt_view(yb_f, 0, 32, 8)
            yb4 = _y_view(yb_f, NXT4, 32, 8)
            # p5 runs inside pass A (its direct matmuls fill tensor idle
            # and cost no vector-engine work) -> static buffers
            x5_f = res_pool.tile([P, 4 * N5], F16, name="x5f")
            v5 = [_pad_view(x5_f, i * N5, 16, 16) for i in range(4)]

            pools = (psum_pool, ot_pool, ya_pool)

            # ---- startup DMAs: first-layer weights + cls Xt0, then rest
            # the very first matmuls' Xt chunks lead the sync queue
            for c in range(NCH):
                nc.sync.dma_start(xtc[:, c, 0, 0:33], xt3_d[c, :, 0, 0:33])
            ww00 = _load_ww(nc, wtw_pool, sww_d, 0, 0, "a", fine=True)
            # remaining Xt0 chunks ride the gpsimd SWDGE queue, parallel to
            # the startup-critical weight DMAs on the sync queue
            for j in range(6):
                for c in range(NCH):
                    if j > 0:
                        nc.gpsimd.dma_start(xtc[:, c, j, 0:33],
                                            xt3_d[c, :, j, 0:33])
                    nc.gpsimd.dma_start(xtc[:, c, j, 33:66],
                                        xt3_d[c, :, j, 33:66])
            for c in range(NCH):
                nc.sync.dma_start(xtb[:, c], xt3_d[c])
            nc.gpsimd.dma_start(
                sbias[:],
                sb_d[:].rearrange("s l a p o -> p (s l a o)")
                       .rearrange("p (s l a o) -> p s l a o",
                                  s=2, l=NL, a=NCH))
            nc.gpsimd.dma_start(pwct[:], pwct_d[:])
            nc.gpsimd.dma_start(pwbt[:], pwbt_d[:])
            nc.gpsimd.dma_start(pwc5[:], pwc5_d[:])
            nc.gpsimd.dma_start(pwb5[:], pwb5_d[:])
            nc.gpsimd.dma_start(pbc_t[0:20], pbc_d[:])
            nc.gpsimd.dma_start(pbb_t[0:5], pbb_d[:])
            for c in range(NCH):
                nc.gpsimd.dma_start(v5[0][:, c], x5_d[c])
            _zero_y_ring(nc, yc, 64, 16)
            _zero_y_ring(nc, yb, 64, 16)
            for i in range(1, 4):
                _zero_ring(nc, v5[i], 16, 16)

            # ---- pass A: p3 winograd + p5 direct, chains interleaved
            # p5 rotation: cls v0->v1->v3->v1->v3 (tower v3);
            #              box v0->v2->v0->v2->v0 (tower v0)
            cls_chain = [(0, 1), (1, 3), (3, 1), (1, 3)]
            box_chain = [(0, 2), (2, 0), (0, 2), (2, 0)]
            ww = {(0, 0): ww00}

            def get_ww(s, l, tag):
                if (s, l) not in ww:
                    ww[(s, l)] = _load_ww(nc, wtw_pool, sww_d, s, l, tag)
                return ww[(s, l)]

            ycp = _pad_view(yc_f, 0, 64, 64)
            ybp = _pad_view(yb_f, 0, 64, 64)
            y4cp = _pad_view(yc_f, NXT4, 32, 32)
            y4bp = _pad_view(yb_f, NXT4, 32, 32)

            for l in range(NL):
                for s, xt, y, chain in ((0, xtc, yc, cls_chain),
                                        (1, xtb, yb, box_chain)):
                    wt = get_ww(s, l, "a")
                    wp = _load_wp(nc, wtp_pool, swp_d, s, l, "a")
                    if l > 0:
                        _intrans(nc, it_pool, y, xt, 64, 16, f"a{s}{l}A",
                                 1, 34)
                        _intrans(nc, it_pool, y, xt, 64, 16, f"a{s}{l}B",
                                 34, 65)
                    if l == NL - 1:
                        # towers in plain layout: preds run as direct convs
                        yp = ycp if s == 0 else ybp
                        _zero_ring(nc, yp, 64, 64)
                        _wino_layer(nc, pools, wt, xt, yp, sbias[:, s, l],
                                    64, 16, f"a{s}{l}", plain=True)
                    else:
                        _wino_layer(nc, pools, wt, xt, y, sbias[:, s, l],
                                    64, 16, f"a{s}{l}")
                    si, di = chain[l]
                    _conv_layer(nc, psum_pool, wp, v5[si], v5[di],
                                sbias[:, s, l], 16, 16, 16, f"a5{s}{l}")
                    if l == NL - 1 and s == 0:
                        # cls-head pred bands run during the box chain's
                        # final layer (its transforms own DVE/gpsimd then)
                        for it in range(8):
                            _preds5_head(nc, psum_pool, stage_pool, pwc5,
                                         pbc_t, ycp, out_d, 64, 64, 8, 0,
                                         0, 20, "a3c", it)
                    # prefetch next (s, l) winograd weights
                    nl_s, nl_l = (1, l) if s == 0 else (0, l + 1)
                    if nl_l < NL:
                        get_ww(nl_s, nl_l, "a")
            for it in range(8):
                _preds5_head(nc, psum_pool, stage_pool, pwb5, pbb_t, ybp,
                             out_d, 64, 64, 8, 0, 20, 5, "a3b", it)
            _preds5(nc, psum_pool, stage_pool, pwc5, pwb5, pbc_t, pbb_t,
                    v5[3], v5[0], out_d, 16, 16, 16, 5120, "a5")
            # p4 staging into regions freed once the p3 preds are done
            for c in range(NCH):
                nc.sync.dma_start(xtc4[:, c], xt4_d[c])
                nc.sync.dma_start(xtb4[:, c], xt4_d[c])
            _zero_y_ring(nc, yc4, 32, 8)
            _zero_y_ring(nc, yb4, 32, 8)

            # ---- pass B: p4 winograd stems + direct preds
            for l in range(NL):
                for s, xt4, y4 in ((0, xtc4, yc4), (1, xtb4, yb4)):
                    # reload winograd weights for pass b (pool rotation)
                    wt = _load_ww(nc, wtw_pool, sww_d, s, l, "b")
                    if l > 0:
                        _intrans(nc, it_pool, y4, xt4, 32, 8, f"b{s}{l}")
                    if l == NL - 1:
                        yp = y4cp if s == 0 else y4bp
                        _zero_ring(nc, yp, 32, 32)
                        _wino_layer(nc, pools, wt, xt4, yp, sbias[:, s, l],
                                    32, 8, f"b{s}{l}", plain=True)
                    else:
                        _wino_layer(nc, pools, wt, xt4, y4, sbias[:, s, l],
                                    32, 8, f"b{s}{l}")
            _preds5(nc, psum_pool, stage_pool, pwc5, pwb5, pbc_t, pbb_t,
                    y4cp, y4bp, out_d, 32, 32, 16, 4096, "b3")

    nc.compile()
    return nc


# ------------------------------------------------------------- host side
def _wino_xt(x):
    """x [256, H, W] fp32 -> Xt [2, 128, 6, H+2, W//4] fp16."""
    Cc, H, W = x.shape
    T = W // 4
    xp = np.pad(x, ((0, 0), (1, 1), (1, 3))).astype(np.float32)
    xa = [xp[:, :, a:a + 4 * T:4] for a in range(6)]
    g = xa[3] - xa[1]
    h = xa[4] - xa[2]
    r3 = g * 2 + h
    r4 = g * -2 + h
    r5 = (xa[5] - xa[3]) + g * -4
    r0 = (xa[0] - xa[2]) * 4 + h
    r2 = (xa[1] - xa[2]) * 4 + (xa[4] - xa[3])
    r1 = (xa[1] + xa[2]) * -4 + (xa[3] + xa[4])
    xt = np.stack([r0, r1, r2, r3, r4, r5])          # [6, 256, H+2, T]
    xt = xt.reshape(6, NCH, P, H + 2, T).transpose(1, 2, 0, 3, 4)
    return np.ascontiguousarray(xt, dtype=np.float16)


def _pack_ww(wcls, wbox):
    """[2][NL, 256, 256, 3, 3] -> [2, NL, 128ip, 2ic, 2oc, 6j, 3k, 128op]."""
    out = np.empty((2, NL, P, NCH, NCH, 6, 3, P), np.float16)
    for s, w in enumerate((wcls, wbox)):
        for l in range(NL):
            t = np.einsum('ja,oika->oikj', G_MAT, w[l].astype(np.float64))
            t = t.reshape(NCH, P, NCH, P, 3, 6).transpose(3, 2, 0, 5, 4, 1)
            out[s, l] = t.astype(np.float16)
    return np.ascontiguousarray(out)


def _pack_pred_wt(w):
    """[n, 256, 3, 3] -> [128ip, 2ic, 6j, 3k, n]."""
    n = w.shape[0]
    t = np.einsum('ja,oika->oikj', G_MAT, w.astype(np.float64))
    t = t.reshape(n, NCH, P, 3, 6).transpose(2, 1, 4, 3, 0)
    return np.ascontiguousarray(t, dtype=np.float16)


def _pack_stem_wp(wcls, wbox):
    w = np.stack([wcls, wbox]).reshape(2, NL, NCH, P, NCH, P, 3, 3)
    w = w.transpose(0, 1, 5, 4, 2, 6, 7, 3)
    return np.ascontiguousarray(w.reshape(2, NL, P, NCH, NCH, 9, P),
                                dtype=np.float16)


def _pack_pred_wp(w):
    n = w.shape[0]
    w = w.reshape(n, NCH, P, 3, 3).transpose(2, 1, 3, 4, 0)
    return np.ascontiguousarray(w.reshape(P, NCH, 9, n), dtype=np.float16)


def kernel(p3, p4, p5, stem_cls_w, stem_cls_b, stem_box_w, stem_box_b,
           pred_cls_w, pred_cls_b, pred_box_w, pred_box_b,
           pred_ctr_w, pred_ctr_b):
    if 'nc' not in _cached:
        _cached['nc'] = _build()
    nc = _cached['nc']

    B = p3.shape[0]
    scw = np.asarray(stem_cls_w, np.float32)
    sbw = np.asarray(stem_box_w, np.float32)
    shared = {
        "sww": _pack_ww(scw, sbw),
        "swp": _pack_stem_wp(scw, sbw),
        "sb": np.ascontiguousarray(
            np.stack([stem_cls_b, stem_box_b]).reshape(2, NL, NCH, P, 1),
            dtype=np.float32),
        "pwct": _pack_pred_wt(np.asarray(pred_cls_w)),
        "pwbt": _pack_pred_wt(
            np.concatenate([pred_box_w, pred_ctr_w], axis=0)),
        "pwc5": _pack_pred_wp(np.asarray(pred_cls_w)),
        "pwb5": _pack_pred_wp(
            np.concatenate([pred_box_w, pred_ctr_w], axis=0)),
        "pbc": np.asarray(pred_cls_b, np.float32).reshape(20, 1),
        "pbb": np.concatenate(
            [pred_box_b, pred_ctr_b]).astype(np.float32).reshape(5, 1),
    }
    in_maps = []
    for b in range(B):
        m = dict(shared)
        m["xt3"] = _wino_xt(np.asarray(p3[b], np.float32))
        m["xt4"] = _wino_xt(np.asarray(p4[b], np.float32))
        m["x5"] = np.pad(
            np.asarray(p5[b], np.float16).reshape(NCH, P, 16, 16),
            ((0, 0), (0, 0), (1, 1), (1, 1)))
        in_maps.append(m)

    res = run_bass_kernel_spmd(nc, in_maps, core_ids=list(range(B)),
                               **_run_opts)
    _last['res'] = res
    out = np.stack([r["out"].T for r in res.results])
    return np.ascontiguousarray(out, dtype=np.float32)


# revision 51
# speedup vs baseline: 1.2330x; 1.2330x over previous
"""FCOS head (nn_FCOS_73787538145418) Trainium2 Bass kernel.

Sharding: data-parallel, one image per NeuronCore (B=8 across 8 cores),
weights replicated. Each core runs the identical SPMD NEFF over its image.

Algorithm: 1D Winograd F(4,3) along W (direct 3-tap conv along H) for the
p3/p4 levels, direct conv for p5. Halves tensor-engine rows for stems and
prediction convs on p3/p4. fp16 operands (1 cyc/row on the PE), fp32 PSUM.
Per conv layer: W-transformed input Xt[j=0..5] (host-computed for the
feature, DVE-computed between layers), 6 PSUM groups m_j accumulated over
(k=3 H-taps x 2 ci chunks), inverse transform y = AT m on DVE/gpsimd,
bias+ReLU on the scalar engine writing a tiled-column spatial layout
[rows, 4, W/4+1] that keeps the next in-transform's reads contiguous.
Output is [25, 5376] channel-major per core; host transposes/stacks.
"""
import sys

if '/opt/trn_rl_repo' not in sys.path:
    sys.path.insert(0, '/opt/trn_rl_repo')

import numpy as np

import concourse.mybir as mybir
from concourse import bacc
import concourse.tile as tile
from concourse.bass_utils import run_bass_kernel_spmd

P = 128
NCH = 2                 # 256 channels = 2 chunks of 128
NL = 4                  # stem depth
NPIX_TOTAL = 5376
F16 = mybir.dt.float16
F32 = mybir.dt.float32
AL = mybir.AluOpType
AF = mybir.ActivationFunctionType

# F(4,3) Winograd (points [0, 1, -1, 2, -2])
G_MAT = np.array([
    [1 / 4, 0, 0], [-1 / 6, -1 / 6, -1 / 6], [-1 / 6, 1 / 6, -1 / 6],
    [1 / 24, 1 / 12, 1 / 6], [1 / 24, -1 / 12, 1 / 6], [0, 0, 1]])

_cached = {}
_run_opts = {}   # extra kwargs for run_bass_kernel_spmd (test harness: trace)
_last = {}       # last BassKernelResults (test harness reads exec_time_ns)


# ---------------------------------------------------------------- views
def _xt_view(flat, off, H, T):
    n = NCH * 6 * (H + 2) * T
    return flat[:, off:off + n].rearrange(
        "p (c j r t) -> p c j r t", c=NCH, j=6, r=H + 2)


def _y_view(flat, off, H, T):
    # tiled-column spatial layout: col = 4*tw + f, tw in [0, T], f in [0, 4)
    n = NCH * (H + 2) * 4 * (T + 1)
    return flat[:, off:off + n].rearrange(
        "p (c r f t) -> p c r f t", c=NCH, r=H + 2, f=4)


def _pad_view(flat, off, H, W):
    n = NCH * (H + 2) * (W + 2)
    return flat[:, off:off + n].rearrange(
        "p (c h w) -> p c h w", c=NCH, h=H + 2, w=W + 2)


def _zero_y_ring(nc, y, H, T):
    nc.vector.memset(y[:, :, 0], 0.0)
    nc.vector.memset(y[:, :, H + 1], 0.0)
    nc.vector.memset(y[:, :, 1:H + 1, 0, 0], 0.0)
    nc.vector.memset(y[:, :, 1:H + 1, 1:4, T], 0.0)


def _zero_ring(nc, v, H, W):
    for c in range(NCH):
        nc.vector.memset(v[:, c, 0, :], 0.0)
        nc.vector.memset(v[:, c, H + 1, :], 0.0)
        nc.vector.memset(v[:, c, 1:H + 1, 0], 0.0)
        nc.vector.memset(v[:, c, 1:H + 1, W + 1], 0.0)


# ------------------------------------------------------- winograd pieces
def _intrans(nc, it_pool, y, xt, H, T, tag, r0=1, r1=None):
    """W-direction F(4,3) input transform: y spatial -> xt[j], rows r0..r1.

    Callers split the row range so the first band's matmuls can start
    after the first chunk instead of the full-image transform."""
    if r1 is None:
        r1 = H + 1
    H = r1 - r0
    xa = [y[:, :, r0:r1, a, 0:T] for a in range(4)]
    xa.append(y[:, :, r0:r1, 0, 1:T + 1])
    xa.append(y[:, :, r0:r1, 1, 1:T + 1])

    def scr(nm):
        return it_pool.tile([P, NCH, H, T], F16, tag="it",
                            name=f"it_{tag}_{nm}")[:]
    V, GP = nc.vector, nc.gpsimd
    g = scr("g"); V.tensor_tensor(g, xa[3], xa[1], AL.subtract)
    h = scr("h"); V.tensor_tensor(h, xa[4], xa[2], AL.subtract)
    V.scalar_tensor_tensor(xt[:, :, 3, r0:r1], g, 2.0, h, AL.mult, AL.add)
    V.scalar_tensor_tensor(xt[:, :, 4, r0:r1], g, -2.0, h, AL.mult, AL.add)
    m = scr("m"); V.tensor_tensor(m, xa[5], xa[3], AL.subtract)
    V.scalar_tensor_tensor(xt[:, :, 5, r0:r1], g, -4.0, m, AL.mult, AL.add)
    f = scr("f"); V.tensor_tensor(f, xa[0], xa[2], AL.subtract)
    V.scalar_tensor_tensor(xt[:, :, 0, r0:r1], f, 4.0, h, AL.mult, AL.add)
    u = scr("u"); V.tensor_tensor(u, xa[1], xa[2], AL.subtract)
    v = scr("v"); V.tensor_tensor(v, xa[4], xa[3], AL.subtract)
    V.scalar_tensor_tensor(xt[:, :, 2, r0:r1], u, 4.0, v, AL.mult, AL.add)
    p_ = scr("p"); V.tensor_tensor(p_, xa[1], xa[2], AL.add)
    q = scr("q"); V.tensor_tensor(q, xa[3], xa[4], AL.add)
    V.scalar_tensor_tensor(xt[:, :, 1, r0:r1], p_, -4.0, q, AL.mult, AL.add)


def _wino_unit(nc, psum_pool, ot_pool, ya_pool, lhsT_fn, xt, r0, R, T,
               nlo, nhi, tag):
    """One band: 6 PSUM groups (3k x 2ci matmuls each) + inverse transform.

    lhsT_fn(c, j, k) -> weight AP [K=128, M]; output written to partitions
    nlo:nhi of psum/scr tiles. Returns yact tile view [nlo:nhi, R, 4, T]."""
    def scr(nm):
        t = ot_pool.tile([P, R, T], F16, tag="ot", name=f"ot_{tag}_{nm}")
        return t[nlo:nhi]

    cs = []
    def mm(j):
        # 6 accumulating matmuls into one PSUM group, then a scalar-engine
        # copy to fp16 SBUF (PSUM allows only one engine-instruction input;
        # the copy also releases the PSUM bank early)
        ps = psum_pool.tile([P, R, T], F32, tag="ps", name=f"ps_{tag}_{j}")
        kk = 0
        for c in range(NCH):
            for k in range(3):
                nc.tensor.matmul(ps[nlo:nhi], lhsT_fn(c, j, k),
                                 xt[:, c, j, r0 + k:r0 + k + R, :],
                                 start=(kk == 0), stop=(kk == 5))
                kk += 1
        cj = scr(f"c{j}")
        nc.scalar.activation(cj, ps[nlo:nhi], AF.Copy)
        cs.append(cj)

    ya = ya_pool.tile([P, R, 4, T], F16, tag="ya", name=f"ya_{tag}")
    V, GP = nc.vector, nc.gpsimd

    mm(0); mm(1); mm(2)
    s = scr("s"); V.tensor_tensor(s, cs[1], cs[2], AL.add)
    d = scr("d"); V.tensor_tensor(d, cs[1], cs[2], AL.subtract)
    mm(3); mm(4)
    # DVE TensorTensor runs in 2x_1p mode (~4x cheaper than gpsimd);
    # gpsimd keeps only S/D so y1/y2 can start while DVE finishes u/y0
    u = scr("u"); V.tensor_tensor(u, cs[0], s, AL.add)
    S = scr("S"); GP.tensor_tensor(S, cs[3], cs[4], AL.add)
    D = scr("D"); GP.tensor_tensor(D, cs[3], cs[4], AL.subtract)
    V.scalar_tensor_tensor(ya[nlo:nhi, :, 1, :], D, 2.0, d, AL.mult, AL.add)
    V.scalar_tensor_tensor(ya[nlo:nhi, :, 2, :], S, 4.0, s, AL.mult, AL.add)
    mm(5)
    V.tensor_tensor(ya[nlo:nhi, :, 0, :], u, S, AL.add)
    v3 = scr("v3")
    V.scalar_tensor_tensor(v3, D, 8.0, d, AL.mult, AL.add)
    V.tensor_tensor(ya[nlo:nhi, :, 3, :], v3, cs[5], AL.add)
    return ya


def _wino_layer(nc, pools, wt, xt, ydst, bias_ap, H, T, tag, plain=False):
    """Full 256->256 W-winograd conv + bias + relu.

    plain=False: ydst is the tiled-column layout (feeds next in-transform).
    plain=True: ydst is a plain padded [c, H+2, W+2] view (feeds direct
    prediction convs); the activation collapses to one instruction."""
    psum_pool, ot_pool, ya_pool = pools
    bands = [(0, 32), (32, 32)] if H == 64 else [(0, H)]
    W = 4 * T
    for bi, (r0, R) in enumerate(bands):
        for o in range(NCH):
            ya = _wino_unit(nc, psum_pool, ot_pool, ya_pool,
                            lambda c, j, k: wt[:, c, o, j, k, :],
                            xt, r0, R, T, 0, P, f"{tag}{bi}{o}")
            rows = slice(r0 + 1, r0 + 1 + R)
            if plain:
                dv = ydst[:, o, rows, 1:W + 1].rearrange(
                    "p r (t f) -> p r f t", f=4)
                nc.scalar.activation(dv, ya[:], AF.Relu, bias=bias_ap[:, o])
            else:
                nc.scalar.activation(ydst[:, o, rows, 1:4, 0:T],
                                     ya[:, :, 0:3, :], AF.Relu,
                                     bias=bias_ap[:, o])
                nc.scalar.activation(ydst[:, o, rows, 0, 1:T + 1],
                                     ya[:, :, 3, :], AF.Relu,
                                     bias=bias_ap[:, o])


def _wino_preds(nc, pools, stage_pool, pwct, pwbt, pbc_t, pbb_t,
                xtc, xtb, out_d, H, T, pix_base, tag):
    """cls(20ch) + box/ctr(5ch) W-winograd pred convs + bias (no relu)."""
    psum_pool, ot_pool, ya_pool = pools
    bands = [(0, 32), (32, 32)] if H == 64 else [(0, H)]
    for bi, (r0, R) in enumerate(bands):
        n = R * T * 4
        c0 = pix_base + r0 * T * 4
        for hi, (pw, nout, xt, pb, olo) in enumerate(
                ((pwct, 20, xtc, pbc_t, 0), (pwbt, 5, xtb, pbb_t, 20))):
            ya = _wino_unit(nc, psum_pool, ot_pool, ya_pool,
                            lambda c, j, k: pw[:, c, j, k, :],
                            xt, r0, R, T, 0, nout, f"{tag}p{bi}{hi}")
            st = stage_pool.tile([32, R, T, 4], F32, tag="st",
                                 name=f"st_{tag}{bi}{hi}")
            stv = st.rearrange("p r t f -> p r f t")
            nc.scalar.activation(stv[0:nout], ya[0:nout], AF.Identity,
                                 bias=pb[0:nout])
            nc.sync.dma_start(
                out_d[olo:olo + nout, c0:c0 + n],
                st[0:nout].rearrange("p r t f -> p (r t f)"))


# ------------------------------------------------------- p5 direct path
def _conv_layer(nc, psum_pool, wt, src, dst, bias_ap, H, W, R, tag):
    n_tiles = H // R
    for o in range(NCH):
        pss = [psum_pool.tile([P, R, W], F32, tag="ps",
                              name=f"ps_{tag}_{o}_{it}")
               for it in range(n_tiles)]
        k = 0
        for c in range(NCH):
            for ky in range(3):
                for kx in range(3):
                    lhsT = wt[:, c, o, ky * 3 + kx, :]
                    for it in range(n_tiles):
                        r0 = it * R
                        rhs = src[:, c, r0 + ky:r0 + ky + R, kx:kx + W]
                        nc.tensor.matmul(pss[it][:], lhsT, rhs,
                                         start=(k == 0), stop=(k == 17))
                    k += 1
        for it in range(n_tiles):
            r0 = it * R
            nc.scalar.activation(dst[:, o, r0 + 1:r0 + 1 + R, 1:W + 1],
                                 pss[it][:], AF.Relu, bias=bias_ap[:, o])


def _preds5_head(nc, psum_pool, stage_pool, pw, pb_t, tower, out_d,
                 H, W, R, pix_base, olo, nout, tag, it):
    """One R-row band of ONE direct prediction head (cls or box/ctr)."""
    r0 = it * R
    ps = psum_pool.tile([P, R, W], F32, tag="ps", name=f"ph_{tag}_{it}")
    k = 0
    for c in range(NCH):
        for ky in range(3):
            for kx in range(3):
                t = ky * 3 + kx
                nc.tensor.matmul(ps[0:nout], pw[:, c, t, :],
                                 tower[:, c, r0 + ky:r0 + ky + R, kx:kx + W],
                                 start=(k == 0), stop=(k == 17))
                k += 1
    st = stage_pool.tile([32, R, W, 1], F32, tag="st", name=f"sh_{tag}_{it}")
    sf = st.rearrange("p r w o -> p (r w o)")
    nc.scalar.activation(sf[0:nout], ps[0:nout].rearrange("p r w -> p (r w)"),
                         AF.Identity, bias=pb_t[0:nout])
    c0 = pix_base + r0 * W
    nc.sync.dma_start(out_d[olo:olo + nout, c0:c0 + R * W], sf[0:nout])


def _preds5(nc, psum_pool, stage_pool, pwc, pwb, pbc_t, pbb_t,
            cls_tower, box_tower, out_d, H, W, R, pix_base, tag):
    n_tiles = H // R
    for it in range(n_tiles):
        r0 = it * R
        ps1 = psum_pool.tile([P, R, W], F32, tag="ps", name=f"pc_{tag}_{it}")
        ps2 = psum_pool.tile([P, R, W], F32, tag="ps", name=f"pb_{tag}_{it}")
        k = 0
        for c in range(NCH):
            for ky in range(3):
                for kx in range(3):
                    t = ky * 3 + kx
                    rc = cls_tower[:, c, r0 + ky:r0 + ky + R, kx:kx + W]
                    rb = box_tower[:, c, r0 + ky:r0 + ky + R, kx:kx + W]
                    nc.tensor.matmul(ps1[0:20], pwc[:, c, t, :], rc,
                                     start=(k == 0), stop=(k == 17))
                    nc.tensor.matmul(ps2[0:5], pwb[:, c, t, :], rb,
                                     start=(k == 0), stop=(k == 17))
                    k += 1
        st = stage_pool.tile([32, R, W, 1], F32, tag="st", name=f"s5_{tag}_{it}")
        sf = st.rearrange("p r w o -> p (r w o)")
        nc.scalar.activation(sf[0:20], ps1[0:20].rearrange("p r w -> p (r w)"),
                             AF.Identity, bias=pbc_t[0:20])
        st2 = stage_pool.tile([32, R, W, 1], F32, tag="st", name=f"s6_{tag}_{it}")
        sf2 = st2.rearrange("p r w o -> p (r w o)")
        nc.scalar.activation(sf2[0:5], ps2[0:5].rearrange("p r w -> p (r w)"),
                             AF.Identity, bias=pbb_t[0:5])
        c0 = pix_base + r0 * W
        nc.sync.dma_start(out_d[0:20, c0:c0 + R * W], sf[0:20])
        nc.sync.dma_start(out_d[20:25, c0:c0 + R * W], sf2[0:5])


# ------------------------------------------------------------ weight DMA
def _load_ww(nc, wtw_pool, sww_d, s, l, tag, fine=False):
    wt = wtw_pool.tile([P, NCH, NCH, 6, 3, P], F16, tag="ww",
                       name=f"ww_{tag}_{s}_{l}")
    if fine:
        # per-(o, j, c) splits so the first matmuls' deps clear quickly
        for o in range(NCH):
            for j in range(6):
                for c in range(NCH):
                    nc.sync.dma_start(wt[:, c, o, j], sww_d[s, l, :, c, o, j])
    else:
        for c in range(NCH):
            for o in range(NCH):
                nc.sync.dma_start(wt[:, c, o], sww_d[s, l, :, c, o])
    return wt


def _load_wp(nc, wtp_pool, swp_d, s, l, tag):
    wt = wtp_pool.tile([P, NCH, NCH, 9, P], F16, tag="wp",
                       name=f"wp_{tag}_{s}_{l}")
    for c in range(NCH):
        for o in range(NCH):
            nc.sync.dma_start(wt[:, c, o], swp_d[s, l, :, c, o])
    return wt


# ------------------------------------------------------------------ build
def _build():
    nc = bacc.Bacc("TRN2", target_bir_lowering=False, debug=False,
                   num_devices=8)

    # p3 Xt0 (host-transformed), shipped once per chain buffer
    xt3_d = nc.dram_tensor("xt3", (NCH, P, 6, 66, 16), F16,
                           kind="ExternalInput")
    xt4_d = nc.dram_tensor("xt4", (NCH, P, 6, 34, 8), F16,
                           kind="ExternalInput")
    x5_d = nc.dram_tensor("x5", (NCH, P, 18, 18), F16, kind="ExternalInput")
    sww_d = nc.dram_tensor("sww", (2, NL, P, NCH, NCH, 6, 3, P), F16,
                           kind="ExternalInput")
    swp_d = nc.dram_tensor("swp", (2, NL, P, NCH, NCH, 9, P), F16,
                           kind="ExternalInput")
    sb_d = nc.dram_tensor("sb", (2, NL, NCH, P, 1), F32, kind="ExternalInput")
    pwct_d = nc.dram_tensor("pwct", (P, NCH, 6, 3, 20), F16,
                            kind="ExternalInput")
    pwbt_d = nc.dram_tensor("pwbt", (P, NCH, 6, 3, 5), F16,
                            kind="ExternalInput")
    pwc5_d = nc.dram_tensor("pwc5", (P, NCH, 9, 20), F16,
                            kind="ExternalInput")
    pwb5_d = nc.dram_tensor("pwb5", (P, NCH, 9, 5), F16,
                            kind="ExternalInput")
    pbc_d = nc.dram_tensor("pbc", (20, 1), F32, kind="ExternalInput")
    pbb_d = nc.dram_tensor("pbb", (5, 1), F32, kind="ExternalInput")
    out_d = nc.dram_tensor("out", (25, NPIX_TOTAL), F32, kind="ExternalOutput")

    NXT3 = NCH * 6 * 66 * 16      # 12672
    NY3 = NCH * 66 * 68           # 8976
    NXT4 = NCH * 6 * 34 * 8       # 3264
    NY4 = NCH * 34 * 36           # 2448
    N5 = NCH * 18 * 18            # 648

    with tile.TileContext(nc) as tc:
        with (
            tc.tile_pool(name="resident", bufs=1) as res_pool,
            tc.tile_pool(name="wtw", bufs=2) as wtw_pool,
            tc.tile_pool(name="wtp", bufs=1) as wtp_pool,
            tc.tile_pool(name="psum", bufs=8, space="PSUM") as psum_pool,
            tc.tile_pool(name="stage", bufs=2) as stage_pool,
            tc.tile_pool(name="it", bufs=5) as it_pool,
            tc.tile_pool(name="ot", bufs=18) as ot_pool,
            tc.tile_pool(name="ya", bufs=4) as ya_pool,
        ):
            xtc_f = res_pool.tile([P, NXT3], F16, name="xtc")
            xtb_f = res_pool.tile([P, NXT3], F16, name="xtb")
            yc_f = res_pool.tile([P, NY3], F16, name="yc")
            yb_f = res_pool.tile([P, NY3], F16, name="yb")

            sbias = res_pool.tile([P, 2, NL, NCH, 1], F32, name="sbias")
            pwct = res_pool.tile([P, NCH, 6, 3, 20], F16, name="pwct")
            pwbt = res_pool.tile([P, NCH, 6, 3, 5], F16, name="pwbt")
            pwc5 = res_pool.tile([P, NCH, 9, 20], F16, name="pwc5")
            pwb5 = res_pool.tile([P, NCH, 9, 5], F16, name="pwb5")
            pbc_t = res_pool.tile([32, 1], F32, name="pbct")
            pbb_t = res_pool.tile([32, 1], F32, name="pbbt")

            xtc = _xt_view(xtc_f, 0, 64, 16)
            xtb = _xt_view(xtb_f, 0, 64, 16)
            yc = _y_view(yc_f, 0, 64, 16)
            yb = _y_view(yb_f, 0, 64, 16)
            # pass-b carvings
            xtc4 = _xt_view(yc_f, 0, 32, 8)
            yc4 = _y_view(yc_f, NXT4, 32, 8)
            xtb4 = _xt_view(yb_f, 0, 32, 8)
            yb4 = _y_view(yb_f, NXT4, 32, 8)
            # p5 runs inside pass A (its direct matmuls fill tensor idle
            # and cost no vector-engine work) -> static buffers
            x5_f = res_pool.tile([P, 4 * N5], F16, name="x5f")
            v5 = [_pad_view(x5_f, i * N5, 16, 16) for i in range(4)]

            pools = (psum_pool, ot_pool, ya_pool)

            # ---- startup DMAs: first-layer weights + cls Xt0, then rest
            # the very first matmuls' Xt chunks lead the sync queue
            for c in range(NCH):
                nc.sync.dma_start(xtc[:, c, 0, 0:33], xt3_d[c, :, 0, 0:33])
            ww00 = _load_ww(nc, wtw_pool, sww_d, 0, 0, "a", fine=True)
            # remaining Xt0 chunks ride the gpsimd SWDGE queue, parallel to
            # the startup-critical weight DMAs on the sync queue
            for j in range(6):
                for c in range(NCH):
                    if j > 0:
                        nc.gpsimd.dma_start(xtc[:, c, j, 0:33],
                                            xt3_d[c, :, j, 0:33])
                    nc.gpsimd.dma_start(xtc[:, c, j, 33:66],
                                        xt3_d[c, :, j, 33:66])
            for c in range(NCH):
                nc.sync.dma_start(xtb[:, c], xt3_d[c])
            nc.gpsimd.dma_start(
                sbias[:],
                sb_d[:].rearrange("s l a p o -> p (s l a o)")
                       .rearrange("p (s l a o) -> p s l a o",
                                  s=2, l=NL, a=NCH))
            nc.gpsimd.dma_start(pwct[:], pwct_d[:])
            nc.gpsimd.dma_start(pwbt[:], pwbt_d[:])
            nc.gpsimd.dma_start(pwc5[:], pwc5_d[:])
            nc.gpsimd.dma_start(pwb5[:], pwb5_d[:])
            nc.gpsimd.dma_start(pbc_t[0:20], pbc_d[:])
            nc.gpsimd.dma_start(pbb_t[0:5], pbb_d[:])
            for c in range(NCH):
                nc.gpsimd.dma_start(v5[0][:, c], x5_d[c])
            _zero_y_ring(nc, yc, 64, 16)
            _zero_y_ring(nc, yb, 64, 16)
            for i in range(1, 4):
                _zero_ring(nc, v5[i], 16, 16)

            # ---- pass A: p3 winograd + p5 direct, chains interleaved
            # p5 rotation: cls v0->v1->v3->v1->v3 (tower v3);
            #              box v0->v2->v0->v2->v0 (tower v0)
            cls_chain = [(0, 1), (1, 3), (3, 1), (1, 3)]
            box_chain = [(0, 2), (2, 0), (0, 2), (2, 0)]
            ww = {(0, 0): ww00}

            def get_ww(s, l, tag):
                if (s, l) not in ww:
                    ww[(s, l)] = _load_ww(nc, wtw_pool, sww_d, s, l, tag)
                return ww[(s, l)]

            ycp = _pad_view(yc_f, 0, 64, 64)
            ybp = _pad_view(yb_f, 0, 64, 64)
            y4cp = _pad_view(yc_f, NXT4, 32, 32)
            y4bp = _pad_view(yb_f, NXT4, 32, 32)

            for l in range(NL):
                for s, xt, y, chain in ((0, xtc, yc, cls_chain),
                                        (1, xtb, yb, box_chain)):
                    wt = get_ww(s, l, "a")
                    wp = _load_wp(nc, wtp_pool, swp_d, s, l, "a")
                    if l > 0:
                        _intrans(nc, it_pool, y, xt, 64, 16, f"a{s}{l}A",
                                 1, 34)
                        _intrans(nc, it_pool, y, xt, 64, 16, f"a{s}{l}B",
                                 34, 65)
                    if l == NL - 1:
                        # towers in plain layout: preds run as direct convs
                        yp = ycp if s == 0 else ybp
                        _zero_ring(nc, yp, 64, 64)
                        _wino_layer(nc, pools, wt, xt, yp, sbias[:, s, l],
                                    64, 16, f"a{s}{l}", plain=True)
                    else:
                        _wino_layer(nc, pools, wt, xt, y, sbias[:, s, l],
                                    64, 16, f"a{s}{l}")
                    si, di = chain[l]
                    _conv_layer(nc, psum_pool, wp, v5[si], v5[di],
                                sbias[:, s, l], 16, 16, 16, f"a5{s}{l}")
                    if l == NL - 1 and s == 0:
                        # cls-head pred bands run during the box chain's
                        # final layer (its transforms own DVE/gpsimd then)
                        for it in range(8):
                            _preds5_head(nc, psum_pool, stage_pool, pwc5,
                                         pbc_t, ycp, out_d, 64, 64, 8, 0,
                                         0, 20, "a3c", it)
                    # prefetch next (s, l) winograd weights
                    nl_s, nl_l = (1, l) if s == 0 else (0, l + 1)
                    if nl_l < NL:
                        get_ww(nl_s, nl_l, "a")
            for it in range(8):
                _preds5_head(nc, psum_pool, stage_pool, pwb5, pbb_t, ybp,
                             out_d, 64, 64, 8, 0, 20, 5, "a3b", it)
            _preds5(nc, psum_pool, stage_pool, pwc5, pwb5, pbc_t, pbb_t,
                    v5[3], v5[0], out_d, 16, 16, 16, 5120, "a5")
            # p4 staging into regions freed once the p3 preds are done
            for c in range(NCH):
                nc.sync.dma_start(xtc4[:, c], xt4_d[c])
                nc.sync.dma_start(xtb4[:, c], xt4_d[c])
            _zero_y_ring(nc, yc4, 32, 8)
            _zero_y_ring(nc, yb4, 32, 8)

            # ---- pass B: p4 winograd stems + direct preds
            for l in range(NL):
                for s, xt4, y4 in ((0, xtc4, yc4), (1, xtb4, yb4)):
                    # reload winograd weights for pass b (pool rotation)
                    wt = _load_ww(nc, wtw_pool, sww_d, s, l, "b")
                    if l > 0:
                        _intrans(nc, it_pool, y4, xt4, 32, 8, f"b{s}{l}")
                    if l == NL - 1:
                        yp = y4cp if s == 0 else y4bp
                        _zero_ring(nc, yp, 32, 32)
                        _wino_layer(nc, pools, wt, xt4, yp, sbias[:, s, l],
                                    32, 8, f"b{s}{l}", plain=True)
                    else:
                        _wino_layer(nc, pools, wt, xt4, y4, sbias[:, s, l],
                                    32, 8, f"b{s}{l}")
            _preds5(nc, psum_pool, stage_pool, pwc5, pwb5, pbc_t, pbb_t,
                    y4cp, y4bp, out_d, 32, 32, 16, 4096, "b3")

    nc.compile()
    return nc


# ------------------------------------------------------------- host side
def _wino_xt(x):
    """x [256, H, W] fp32 -> Xt [2, 128, 6, H+2, W//4] fp16."""
    Cc, H, W = x.shape
    T = W // 4
    xp = np.pad(x, ((0, 0), (1, 1), (1, 3))).astype(np.float32)
    xa = [xp[:, :, a:a + 4 * T:4] for a in range(6)]
    g = xa[3] - xa[1]
    h = xa[4] - xa[2]
    r3 = g * 2 + h
    r4 = g * -2 + h
    r5 = (xa[5] - xa[3]) + g * -4
    r0 = (xa[0] - xa[2]) * 4 + h
    r2 = (xa[1] - xa[2]) * 4 + (xa[4] - xa[3])
    r1 = (xa[1] + xa[2]) * -4 + (xa[3] + xa[4])
    xt = np.stack([r0, r1, r2, r3, r4, r5])          # [6, 256, H+2, T]
    xt = xt.reshape(6, NCH, P, H + 2, T).transpose(1, 2, 0, 3, 4)
    return np.ascontiguousarray(xt, dtype=np.float16)


def _pack_ww(wcls, wbox):
    """[2][NL, 256, 256, 3, 3] -> [2, NL, 128ip, 2ic, 2oc, 6j, 3k, 128op]."""
    out = np.empty((2, NL, P, NCH, NCH, 6, 3, P), np.float16)
    for s, w in enumerate((wcls, wbox)):
        for l in range(NL):
            t = np.einsum('ja,oika->oikj', G_MAT, w[l].astype(np.float64))
            t = t.reshape(NCH, P, NCH, P, 3, 6).transpose(3, 2, 0, 5, 4, 1)
            out[s, l] = t.astype(np.float16)
    return np.ascontiguousarray(out)


def _pack_pred_wt(w):
    """[n, 256, 3, 3] -> [128ip, 2ic, 6j, 3k, n]."""
    n = w.shape[0]
    t = np.einsum('ja,oika->oikj', G_MAT, w.astype(np.float64))
    t = t.reshape(n, NCH, P, 3, 6).transpose(2, 1, 4, 3, 0)
    return np.ascontiguousarray(t, dtype=np.float16)


def _pack_stem_wp(wcls, wbox):
    w = np.stack([wcls, wbox]).reshape(2, NL, NCH, P, NCH, P, 3, 3)
    w = w.transpose(0, 1, 5, 4, 2, 6, 7, 3)
    return np.ascontiguousarray(w.reshape(2, NL, P, NCH, NCH, 9, P),
                                dtype=np.float16)


def _pack_pred_wp(w):
    n = w.shape[0]
    w = w.reshape(n, NCH, P, 3, 3).transpose(2, 1, 3, 4, 0)
    return np.ascontiguousarray(w.reshape(P, NCH, 9, n), dtype=np.float16)


def kernel(p3, p4, p5, stem_cls_w, stem_cls_b, stem_box_w, stem_box_b,
           pred_cls_w, pred_cls_b, pred_box_w, pred_box_b,
           pred_ctr_w, pred_ctr_b):
    if 'nc' not in _cached:
        _cached['nc'] = _build()
    nc = _cached['nc']

    B = p3.shape[0]
    scw = np.asarray(stem_cls_w, np.float32)
    sbw = np.asarray(stem_box_w, np.float32)
    shared = {
        "sww": _pack_ww(scw, sbw),
        "swp": _pack_stem_wp(scw, sbw),
        "sb": np.ascontiguousarray(
            np.stack([stem_cls_b, stem_box_b]).reshape(2, NL, NCH, P, 1),
            dtype=np.float32),
        "pwct": _pack_pred_wt(np.asarray(pred_cls_w)),
        "pwbt": _pack_pred_wt(
            np.concatenate([pred_box_w, pred_ctr_w], axis=0)),
        "pwc5": _pack_pred_wp(np.asarray(pred_cls_w)),
        "pwb5": _pack_pred_wp(
            np.concatenate([pred_box_w, pred_ctr_w], axis=0)),
        "pbc": np.asarray(pred_cls_b, np.float32).reshape(20, 1),
        "pbb": np.concatenate(
            [pred_box_b, pred_ctr_b]).astype(np.float32).reshape(5, 1),
    }
    in_maps = []
    for b in range(B):
        m = dict(shared)
        m["xt3"] = _wino_xt(np.asarray(p3[b], np.float32))
        m["xt4"] = _wino_xt(np.asarray(p4[b], np.float32))
        m["x5"] = np.pad(
            np.asarray(p5[b], np.float16).reshape(NCH, P, 16, 16),
            ((0, 0), (0, 0), (1, 1), (1, 1)))
        in_maps.append(m)

    res = run_bass_kernel_spmd(nc, in_maps, core_ids=list(range(B)),
                               **_run_opts)
    _last['res'] = res
    out = np.stack([r["out"].T for r in res.results])
    return np.ascontiguousarray(out, dtype=np.float32)


# revision 52
# speedup vs baseline: 1.3818x; 1.1207x over previous
"""FCOS head (nn_FCOS_73787538145418) Trainium2 Bass kernel.

Sharding: data-parallel, one image per NeuronCore (B=8 across 8 cores),
weights replicated. Each core runs the identical SPMD NEFF over its image.

Algorithm: 1D Winograd F(4,3) along W (direct 3-tap conv along H) for the
p3/p4 levels, direct conv for p5. Halves tensor-engine rows for stems and
prediction convs on p3/p4. fp16 operands (1 cyc/row on the PE), fp32 PSUM.
Per conv layer: W-transformed input Xt[j=0..5] (host-computed for the
feature, DVE-computed between layers), 6 PSUM groups m_j accumulated over
(k=3 H-taps x 2 ci chunks), inverse transform y = AT m on DVE/gpsimd,
bias+ReLU on the scalar engine writing a tiled-column spatial layout
[rows, 4, W/4+1] that keeps the next in-transform's reads contiguous.
Output is [25, 5376] channel-major per core; host transposes/stacks.
"""
import sys

if '/opt/trn_rl_repo' not in sys.path:
    sys.path.insert(0, '/opt/trn_rl_repo')

import numpy as np

import concourse.mybir as mybir
from concourse import bacc
import concourse.tile as tile
from concourse.bass_utils import run_bass_kernel_spmd

P = 128
NCH = 2                 # 256 channels = 2 chunks of 128
NL = 4                  # stem depth
NPIX_TOTAL = 5376
F16 = mybir.dt.float16
F32 = mybir.dt.float32
AL = mybir.AluOpType
AF = mybir.ActivationFunctionType

# F(4,3) Winograd (points [0, 1, -1, 2, -2])
G_MAT = np.array([
    [1 / 4, 0, 0], [-1 / 6, -1 / 6, -1 / 6], [-1 / 6, 1 / 6, -1 / 6],
    [1 / 24, 1 / 12, 1 / 6], [1 / 24, -1 / 12, 1 / 6], [0, 0, 1]])

_cached = {}
_run_opts = {}   # extra kwargs for run_bass_kernel_spmd (test harness: trace)
_last = {}       # last BassKernelResults (test harness reads exec_time_ns)


# ---------------------------------------------------------------- views
def _xt_view(flat, off, H, T):
    n = NCH * 6 * (H + 2) * T
    return flat[:, off:off + n].rearrange(
        "p (c j r t) -> p c j r t", c=NCH, j=6, r=H + 2)


def _y_view(flat, off, H, T):
    # tiled-column spatial layout: col = 4*tw + f, tw in [0, T], f in [0, 4)
    n = NCH * (H + 2) * 4 * (T + 1)
    return flat[:, off:off + n].rearrange(
        "p (c r f t) -> p c r f t", c=NCH, r=H + 2, f=4)


def _pad_view(flat, off, H, W):
    n = NCH * (H + 2) * (W + 2)
    return flat[:, off:off + n].rearrange(
        "p (c h w) -> p c h w", c=NCH, h=H + 2, w=W + 2)


def _zero_y_ring(nc, y, H, T):
    nc.vector.memset(y[:, :, 0], 0.0)
    nc.vector.memset(y[:, :, H + 1], 0.0)
    nc.vector.memset(y[:, :, 1:H + 1, 0, 0], 0.0)
    nc.vector.memset(y[:, :, 1:H + 1, 1:4, T], 0.0)


def _zero_ring(nc, v, H, W):
    for c in range(NCH):
        nc.vector.memset(v[:, c, 0, :], 0.0)
        nc.vector.memset(v[:, c, H + 1, :], 0.0)
        nc.vector.memset(v[:, c, 1:H + 1, 0], 0.0)
        nc.vector.memset(v[:, c, 1:H + 1, W + 1], 0.0)


# ------------------------------------------------------- winograd pieces
def _intrans(nc, it_pool, y, xt, H, T, tag, r0=1, r1=None):
    """W-direction F(4,3) input transform: y spatial -> xt[j], rows r0..r1.

    Callers split the row range so the first band's matmuls can start
    after the first chunk instead of the full-image transform."""
    if r1 is None:
        r1 = H + 1
    H = r1 - r0
    xa = [y[:, :, r0:r1, a, 0:T] for a in range(4)]
    xa.append(y[:, :, r0:r1, 0, 1:T + 1])
    xa.append(y[:, :, r0:r1, 1, 1:T + 1])

    def scr(nm):
        return it_pool.tile([P, NCH, H, T], F16, tag="it",
                            name=f"it_{tag}_{nm}")[:]
    V, GP = nc.vector, nc.gpsimd
    g = scr("g"); V.tensor_tensor(g, xa[3], xa[1], AL.subtract)
    h = scr("h"); V.tensor_tensor(h, xa[4], xa[2], AL.subtract)
    V.scalar_tensor_tensor(xt[:, :, 3, r0:r1], g, 2.0, h, AL.mult, AL.add)
    V.scalar_tensor_tensor(xt[:, :, 4, r0:r1], g, -2.0, h, AL.mult, AL.add)
    m = scr("m"); V.tensor_tensor(m, xa[5], xa[3], AL.subtract)
    V.scalar_tensor_tensor(xt[:, :, 5, r0:r1], g, -4.0, m, AL.mult, AL.add)
    f = scr("f"); V.tensor_tensor(f, xa[0], xa[2], AL.subtract)
    V.scalar_tensor_tensor(xt[:, :, 0, r0:r1], f, 4.0, h, AL.mult, AL.add)
    u = scr("u"); V.tensor_tensor(u, xa[1], xa[2], AL.subtract)
    v = scr("v"); V.tensor_tensor(v, xa[4], xa[3], AL.subtract)
    V.scalar_tensor_tensor(xt[:, :, 2, r0:r1], u, 4.0, v, AL.mult, AL.add)
    p_ = scr("p"); V.tensor_tensor(p_, xa[1], xa[2], AL.add)
    q = scr("q"); V.tensor_tensor(q, xa[3], xa[4], AL.add)
    V.scalar_tensor_tensor(xt[:, :, 1, r0:r1], p_, -4.0, q, AL.mult, AL.add)


def _wino_unit(nc, psum_pool, ot_pool, ya_pool, lhsT_fn, xt, r0, R, T,
               nlo, nhi, tag):
    """One band: 6 PSUM groups (3k x 2ci matmuls each) + inverse transform.

    lhsT_fn(c, j, k) -> weight AP [K=128, M]; output written to partitions
    nlo:nhi of psum/scr tiles. Returns yact tile view [nlo:nhi, R, 4, T]."""
    def scr(nm):
        t = ot_pool.tile([P, R, T], F16, tag="ot", name=f"ot_{tag}_{nm}")
        return t[nlo:nhi]

    cs = []
    def mm(j):
        # 6 accumulating matmuls into one PSUM group, then a scalar-engine
        # copy to fp16 SBUF (PSUM allows only one engine-instruction input;
        # the copy also releases the PSUM bank early)
        ps = psum_pool.tile([P, R, T], F32, tag="ps", name=f"ps_{tag}_{j}")
        kk = 0
        for c in range(NCH):
            for k in range(3):
                nc.tensor.matmul(ps[nlo:nhi], lhsT_fn(c, j, k),
                                 xt[:, c, j, r0 + k:r0 + k + R, :],
                                 start=(kk == 0), stop=(kk == 5))
                kk += 1
        cj = scr(f"c{j}")
        nc.scalar.activation(cj, ps[nlo:nhi], AF.Copy)
        cs.append(cj)

    ya = ya_pool.tile([P, R, 4, T], F16, tag="ya", name=f"ya_{tag}")
    V, GP = nc.vector, nc.gpsimd

    mm(0); mm(1); mm(2)
    s = scr("s"); V.tensor_tensor(s, cs[1], cs[2], AL.add)
    d = scr("d"); V.tensor_tensor(d, cs[1], cs[2], AL.subtract)
    mm(3); mm(4)
    # DVE TensorTensor runs in 2x_1p mode (~4x cheaper than gpsimd);
    # gpsimd keeps only S/D so y1/y2 can start while DVE finishes u/y0
    u = scr("u"); V.tensor_tensor(u, cs[0], s, AL.add)
    S = scr("S"); V.tensor_tensor(S, cs[3], cs[4], AL.add)
    D = scr("D"); V.tensor_tensor(D, cs[3], cs[4], AL.subtract)
    V.scalar_tensor_tensor(ya[nlo:nhi, :, 1, :], D, 2.0, d, AL.mult, AL.add)
    V.scalar_tensor_tensor(ya[nlo:nhi, :, 2, :], S, 4.0, s, AL.mult, AL.add)
    mm(5)
    V.tensor_tensor(ya[nlo:nhi, :, 0, :], u, S, AL.add)
    v3 = scr("v3")
    V.scalar_tensor_tensor(v3, D, 8.0, d, AL.mult, AL.add)
    V.tensor_tensor(ya[nlo:nhi, :, 3, :], v3, cs[5], AL.add)
    return ya


def _wino_layer(nc, pools, wt, xt, ydst, bias_ap, H, T, tag, plain=False):
    """Full 256->256 W-winograd conv + bias + relu.

    plain=False: ydst is the tiled-column layout (feeds next in-transform).
    plain=True: ydst is a plain padded [c, H+2, W+2] view (feeds direct
    prediction convs); the activation collapses to one instruction."""
    psum_pool, ot_pool, ya_pool = pools
    bands = [(0, 32), (32, 32)] if H == 64 else [(0, H)]
    W = 4 * T
    for bi, (r0, R) in enumerate(bands):
        for o in range(NCH):
            ya = _wino_unit(nc, psum_pool, ot_pool, ya_pool,
                            lambda c, j, k: wt[:, c, o, j, k, :],
                            xt, r0, R, T, 0, P, f"{tag}{bi}{o}")
            rows = slice(r0 + 1, r0 + 1 + R)
            if plain:
                dv = ydst[:, o, rows, 1:W + 1].rearrange(
                    "p r (t f) -> p r f t", f=4)
                nc.scalar.activation(dv, ya[:], AF.Relu, bias=bias_ap[:, o])
            else:
                nc.scalar.activation(ydst[:, o, rows, 1:4, 0:T],
                                     ya[:, :, 0:3, :], AF.Relu,
                                     bias=bias_ap[:, o])
                nc.scalar.activation(ydst[:, o, rows, 0, 1:T + 1],
                                     ya[:, :, 3, :], AF.Relu,
                                     bias=bias_ap[:, o])


def _wino_preds(nc, pools, stage_pool, pwct, pwbt, pbc_t, pbb_t,
                xtc, xtb, out_d, H, T, pix_base, tag):
    """cls(20ch) + box/ctr(5ch) W-winograd pred convs + bias (no relu)."""
    psum_pool, ot_pool, ya_pool = pools
    bands = [(0, 32), (32, 32)] if H == 64 else [(0, H)]
    for bi, (r0, R) in enumerate(bands):
        n = R * T * 4
        c0 = pix_base + r0 * T * 4
        for hi, (pw, nout, xt, pb, olo) in enumerate(
                ((pwct, 20, xtc, pbc_t, 0), (pwbt, 5, xtb, pbb_t, 20))):
            ya = _wino_unit(nc, psum_pool, ot_pool, ya_pool,
                            lambda c, j, k: pw[:, c, j, k, :],
                            xt, r0, R, T, 0, nout, f"{tag}p{bi}{hi}")
            st = stage_pool.tile([32, R, T, 4], F32, tag="st",
                                 name=f"st_{tag}{bi}{hi}")
            stv = st.rearrange("p r t f -> p r f t")
            nc.scalar.activation(stv[0:nout], ya[0:nout], AF.Identity,
                                 bias=pb[0:nout])
            nc.sync.dma_start(
                out_d[olo:olo + nout, c0:c0 + n],
                st[0:nout].rearrange("p r t f -> p (r t f)"))


# ------------------------------------------------------- p5 direct path
def _conv_layer(nc, psum_pool, wt, src, dst, bias_ap, H, W, R, tag):
    n_tiles = H // R
    for o in range(NCH):
        pss = [psum_pool.tile([P, R, W], F32, tag="ps",
                              name=f"ps_{tag}_{o}_{it}")
               for it in range(n_tiles)]
        k = 0
        for c in range(NCH):
            for ky in range(3):
                for kx in range(3):
                    lhsT = wt[:, c, o, ky * 3 + kx, :]
                    for it in range(n_tiles):
                        r0 = it * R
                        rhs = src[:, c, r0 + ky:r0 + ky + R, kx:kx + W]
                        nc.tensor.matmul(pss[it][:], lhsT, rhs,
                                         start=(k == 0), stop=(k == 17))
                    k += 1
        for it in range(n_tiles):
            r0 = it * R
            nc.scalar.activation(dst[:, o, r0 + 1:r0 + 1 + R, 1:W + 1],
                                 pss[it][:], AF.Relu, bias=bias_ap[:, o])


def _preds5_head(nc, psum_pool, stage_pool, pw, pb_t, tower, out_d,
                 H, W, R, pix_base, olo, nout, tag, it):
    """One R-row band of ONE direct prediction head (cls or box/ctr)."""
    r0 = it * R
    ps = psum_pool.tile([P, R, W], F32, tag="ps", name=f"ph_{tag}_{it}")
    k = 0
    for c in range(NCH):
        for ky in range(3):
            for kx in range(3):
                t = ky * 3 + kx
                nc.tensor.matmul(ps[0:nout], pw[:, c, t, :],
                                 tower[:, c, r0 + ky:r0 + ky + R, kx:kx + W],
                                 start=(k == 0), stop=(k == 17))
                k += 1
    st = stage_pool.tile([32, R, W, 1], F32, tag="st", name=f"sh_{tag}_{it}")
    sf = st.rearrange("p r w o -> p (r w o)")
    nc.scalar.activation(sf[0:nout], ps[0:nout].rearrange("p r w -> p (r w)"),
                         AF.Identity, bias=pb_t[0:nout])
    c0 = pix_base + r0 * W
    nc.sync.dma_start(out_d[olo:olo + nout, c0:c0 + R * W], sf[0:nout])


def _preds5(nc, psum_pool, stage_pool, pwc, pwb, pbc_t, pbb_t,
            cls_tower, box_tower, out_d, H, W, R, pix_base, tag):
    n_tiles = H // R
    for it in range(n_tiles):
        r0 = it * R
        ps1 = psum_pool.tile([P, R, W], F32, tag="ps", name=f"pc_{tag}_{it}")
        ps2 = psum_pool.tile([P, R, W], F32, tag="ps", name=f"pb_{tag}_{it}")
        k = 0
        for c in range(NCH):
            for ky in range(3):
                for kx in range(3):
                    t = ky * 3 + kx
                    rc = cls_tower[:, c, r0 + ky:r0 + ky + R, kx:kx + W]
                    rb = box_tower[:, c, r0 + ky:r0 + ky + R, kx:kx + W]
                    nc.tensor.matmul(ps1[0:20], pwc[:, c, t, :], rc,
                                     start=(k == 0), stop=(k == 17))
                    nc.tensor.matmul(ps2[0:5], pwb[:, c, t, :], rb,
                                     start=(k == 0), stop=(k == 17))
                    k += 1
        st = stage_pool.tile([32, R, W, 1], F32, tag="st", name=f"s5_{tag}_{it}")
        sf = st.rearrange("p r w o -> p (r w o)")
        nc.scalar.activation(sf[0:20], ps1[0:20].rearrange("p r w -> p (r w)"),
                             AF.Identity, bias=pbc_t[0:20])
        st2 = stage_pool.tile([32, R, W, 1], F32, tag="st", name=f"s6_{tag}_{it}")
        sf2 = st2.rearrange("p r w o -> p (r w o)")
        nc.scalar.activation(sf2[0:5], ps2[0:5].rearrange("p r w -> p (r w)"),
                             AF.Identity, bias=pbb_t[0:5])
        c0 = pix_base + r0 * W
        nc.sync.dma_start(out_d[0:20, c0:c0 + R * W], sf[0:20])
        nc.sync.dma_start(out_d[20:25, c0:c0 + R * W], sf2[0:5])


# ------------------------------------------------------------ weight DMA
def _load_ww(nc, wtw_pool, sww_d, s, l, tag, fine=False):
    wt = wtw_pool.tile([P, NCH, NCH, 6, 3, P], F16, tag="ww",
                       name=f"ww_{tag}_{s}_{l}")
    if fine:
        # per-(o, j, c) splits so the first matmuls' deps clear quickly
        for o in range(NCH):
            for j in range(6):
                for c in range(NCH):
                    nc.sync.dma_start(wt[:, c, o, j], sww_d[s, l, :, c, o, j])
    else:
        for c in range(NCH):
            for o in range(NCH):
                nc.sync.dma_start(wt[:, c, o], sww_d[s, l, :, c, o])
    return wt


def _load_wp(nc, wtp_pool, swp_d, s, l, tag):
    wt = wtp_pool.tile([P, NCH, NCH, 9, P], F16, tag="wp",
                       name=f"wp_{tag}_{s}_{l}")
    for c in range(NCH):
        for o in range(NCH):
            nc.sync.dma_start(wt[:, c, o], swp_d[s, l, :, c, o])
    return wt


# ------------------------------------------------------------------ build
def _build():
    nc = bacc.Bacc("TRN2", target_bir_lowering=False, debug=False,
                   num_devices=8)

    # p3 Xt0 (host-transformed), shipped once per chain buffer
    xt3_d = nc.dram_tensor("xt3", (NCH, P, 6, 66, 16), F16,
                           kind="ExternalInput")
    xt4_d = nc.dram_tensor("xt4", (NCH, P, 6, 34, 8), F16,
                           kind="ExternalInput")
    x5_d = nc.dram_tensor("x5", (NCH, P, 18, 18), F16, kind="ExternalInput")
    sww_d = nc.dram_tensor("sww", (2, NL, P, NCH, NCH, 6, 3, P), F16,
                           kind="ExternalInput")
    swp_d = nc.dram_tensor("swp", (2, NL, P, NCH, NCH, 9, P), F16,
                           kind="ExternalInput")
    sb_d = nc.dram_tensor("sb", (2, NL, NCH, P, 1), F32, kind="ExternalInput")
    pwct_d = nc.dram_tensor("pwct", (P, NCH, 6, 3, 20), F16,
                            kind="ExternalInput")
    pwbt_d = nc.dram_tensor("pwbt", (P, NCH, 6, 3, 5), F16,
                            kind="ExternalInput")
    pwc5_d = nc.dram_tensor("pwc5", (P, NCH, 9, 20), F16,
                            kind="ExternalInput")
    pwb5_d = nc.dram_tensor("pwb5", (P, NCH, 9, 5), F16,
                            kind="ExternalInput")
    pbc_d = nc.dram_tensor("pbc", (20, 1), F32, kind="ExternalInput")
    pbb_d = nc.dram_tensor("pbb", (5, 1), F32, kind="ExternalInput")
    out_d = nc.dram_tensor("out", (25, NPIX_TOTAL), F32, kind="ExternalOutput")

    NXT3 = NCH * 6 * 66 * 16      # 12672
    NY3 = NCH * 66 * 68           # 8976
    NXT4 = NCH * 6 * 34 * 8       # 3264
    NY4 = NCH * 34 * 36           # 2448
    N5 = NCH * 18 * 18            # 648

    with tile.TileContext(nc) as tc:
        with (
            tc.tile_pool(name="resident", bufs=1) as res_pool,
            tc.tile_pool(name="wtw", bufs=2) as wtw_pool,
            tc.tile_pool(name="wtp", bufs=1) as wtp_pool,
            tc.tile_pool(name="psum", bufs=8, space="PSUM") as psum_pool,
            tc.tile_pool(name="stage", bufs=2) as stage_pool,
            tc.tile_pool(name="it", bufs=5) as it_pool,
            tc.tile_pool(name="ot", bufs=18) as ot_pool,
            tc.tile_pool(name="ya", bufs=4) as ya_pool,
        ):
            xtc_f = res_pool.tile([P, NXT3], F16, name="xtc")
            xtb_f = res_pool.tile([P, NXT3], F16, name="xtb")
            yc_f = res_pool.tile([P, NY3], F16, name="yc")
            yb_f = res_pool.tile([P, NY3], F16, name="yb")

            sbias = res_pool.tile([P, 2, NL, NCH, 1], F32, name="sbias")
            pwct = res_pool.tile([P, NCH, 6, 3, 20], F16, name="pwct")
            pwbt = res_pool.tile([P, NCH, 6, 3, 5], F16, name="pwbt")
            pwc5 = res_pool.tile([P, NCH, 9, 20], F16, name="pwc5")
            pwb5 = res_pool.tile([P, NCH, 9, 5], F16, name="pwb5")
            pbc_t = res_pool.tile([32, 1], F32, name="pbct")
            pbb_t = res_pool.tile([32, 1], F32, name="pbbt")

            xtc = _xt_view(xtc_f, 0, 64, 16)
            xtb = _xt_view(xtb_f, 0, 64, 16)
            yc = _y_view(yc_f, 0, 64, 16)
            yb = _y_view(yb_f, 0, 64, 16)
            # pass-b carvings
            xtc4 = _xt_view(yc_f, 0, 32, 8)
            yc4 = _y_view(yc_f, NXT4, 32, 8)
            xtb4 = _xt_view(yb_f, 0, 32, 8)
            yb4 = _y_view(yb_f, NXT4, 32, 8)
            # p5 runs inside pass A (its direct matmuls fill tensor idle
            # and cost no vector-engine work) -> static buffers
            x5_f = res_pool.tile([P, 4 * N5], F16, name="x5f")
            v5 = [_pad_view(x5_f, i * N5, 16, 16) for i in range(4)]

            pools = (psum_pool, ot_pool, ya_pool)

            # ---- startup DMAs: first-layer weights + cls Xt0, then rest
            # the very first matmuls' Xt chunks lead the sync queue
            for c in range(NCH):
                nc.sync.dma_start(xtc[:, c, 0, 0:33], xt3_d[c, :, 0, 0:33])
            ww00 = _load_ww(nc, wtw_pool, sww_d, 0, 0, "a", fine=True)
            # remaining Xt0 chunks ride the gpsimd SWDGE queue, parallel to
            # the startup-critical weight DMAs on the sync queue
            for j in range(6):
                for c in range(NCH):
                    if j > 0:
                        nc.gpsimd.dma_start(xtc[:, c, j, 0:33],
                                            xt3_d[c, :, j, 0:33])
                    nc.gpsimd.dma_start(xtc[:, c, j, 33:66],
                                        xt3_d[c, :, j, 33:66])
            for c in range(NCH):
                nc.sync.dma_start(xtb[:, c], xt3_d[c])
            nc.gpsimd.dma_start(
                sbias[:],
                sb_d[:].rearrange("s l a p o -> p (s l a o)")
                       .rearrange("p (s l a o) -> p s l a o",
                                  s=2, l=NL, a=NCH))
            nc.gpsimd.dma_start(pwct[:], pwct_d[:])
            nc.gpsimd.dma_start(pwbt[:], pwbt_d[:])
            nc.gpsimd.dma_start(pwc5[:], pwc5_d[:])
            nc.gpsimd.dma_start(pwb5[:], pwb5_d[:])
            nc.gpsimd.dma_start(pbc_t[0:20], pbc_d[:])
            nc.gpsimd.dma_start(pbb_t[0:5], pbb_d[:])
            for c in range(NCH):
                nc.gpsimd.dma_start(v5[0][:, c], x5_d[c])
            _zero_y_ring(nc, yc, 64, 16)
            _zero_y_ring(nc, yb, 64, 16)
            for i in range(1, 4):
                _zero_ring(nc, v5[i], 16, 16)

            # ---- pass A: p3 winograd + p5 direct, chains interleaved
            # p5 rotation: cls v0->v1->v3->v1->v3 (tower v3);
            #              box v0->v2->v0->v2->v0 (tower v0)
            cls_chain = [(0, 1), (1, 3), (3, 1), (1, 3)]
            box_chain = [(0, 2), (2, 0), (0, 2), (2, 0)]
            ww = {(0, 0): ww00}

            def get_ww(s, l, tag):
                if (s, l) not in ww:
                    ww[(s, l)] = _load_ww(nc, wtw_pool, sww_d, s, l, tag)
                return ww[(s, l)]

            ycp = _pad_view(yc_f, 0, 64, 64)
            ybp = _pad_view(yb_f, 0, 64, 64)
            y4cp = _pad_view(yc_f, NXT4, 32, 32)
            y4bp = _pad_view(yb_f, NXT4, 32, 32)

            for l in range(NL):
                for s, xt, y, chain in ((0, xtc, yc, cls_chain),
                                        (1, xtb, yb, box_chain)):
                    wt = get_ww(s, l, "a")
                    wp = _load_wp(nc, wtp_pool, swp_d, s, l, "a")
                    if l > 0:
                        _intrans(nc, it_pool, y, xt, 64, 16, f"a{s}{l}A",
                                 1, 34)
                        _intrans(nc, it_pool, y, xt, 64, 16, f"a{s}{l}B",
                                 34, 65)
                    if l == NL - 1:
                        # towers in plain layout: preds run as direct convs
                        yp = ycp if s == 0 else ybp
                        _zero_ring(nc, yp, 64, 64)
                        _wino_layer(nc, pools, wt, xt, yp, sbias[:, s, l],
                                    64, 16, f"a{s}{l}", plain=True)
                    else:
                        _wino_layer(nc, pools, wt, xt, y, sbias[:, s, l],
                                    64, 16, f"a{s}{l}")
                    si, di = chain[l]
                    _conv_layer(nc, psum_pool, wp, v5[si], v5[di],
                                sbias[:, s, l], 16, 16, 16, f"a5{s}{l}")
                    if l == NL - 1 and s == 0:
                        # cls-head pred bands run during the box chain's
                        # final layer (its transforms own DVE/gpsimd then)
                        for it in range(8):
                            _preds5_head(nc, psum_pool, stage_pool, pwc5,
                                         pbc_t, ycp, out_d, 64, 64, 8, 0,
                                         0, 20, "a3c", it)
                    # prefetch next (s, l) winograd weights
                    nl_s, nl_l = (1, l) if s == 0 else (0, l + 1)
                    if nl_l < NL:
                        get_ww(nl_s, nl_l, "a")
            for it in range(8):
                _preds5_head(nc, psum_pool, stage_pool, pwb5, pbb_t, ybp,
                             out_d, 64, 64, 8, 0, 20, 5, "a3b", it)
            _preds5(nc, psum_pool, stage_pool, pwc5, pwb5, pbc_t, pbb_t,
                    v5[3], v5[0], out_d, 16, 16, 16, 5120, "a5")
            # p4 staging into regions freed once the p3 preds are done
            for c in range(NCH):
                nc.sync.dma_start(xtc4[:, c], xt4_d[c])
                nc.sync.dma_start(xtb4[:, c], xt4_d[c])
            _zero_y_ring(nc, yc4, 32, 8)
            _zero_y_ring(nc, yb4, 32, 8)

            # ---- pass B: p4 winograd stems + direct preds
            for l in range(NL):
                for s, xt4, y4 in ((0, xtc4, yc4), (1, xtb4, yb4)):
                    # reload winograd weights for pass b (pool rotation)
                    wt = _load_ww(nc, wtw_pool, sww_d, s, l, "b")
                    if l > 0:
                        _intrans(nc, it_pool, y4, xt4, 32, 8, f"b{s}{l}")
                    if l == NL - 1:
                        yp = y4cp if s == 0 else y4bp
                        _zero_ring(nc, yp, 32, 32)
                        _wino_layer(nc, pools, wt, xt4, yp, sbias[:, s, l],
                                    32, 8, f"b{s}{l}", plain=True)
                    else:
                        _wino_layer(nc, pools, wt, xt4, y4, sbias[:, s, l],
                                    32, 8, f"b{s}{l}")
            _preds5(nc, psum_pool, stage_pool, pwc5, pwb5, pbc_t, pbb_t,
                    y4cp, y4bp, out_d, 32, 32, 16, 4096, "b3")

    nc.compile()
    return nc


# ------------------------------------------------------------- host side
def _wino_xt(x):
    """x [256, H, W] fp32 -> Xt [2, 128, 6, H+2, W//4] fp16."""
    Cc, H, W = x.shape
    T = W // 4
    xp = np.pad(x, ((0, 0), (1, 1), (1, 3))).astype(np.float32)
    xa = [xp[:, :, a:a + 4 * T:4] for a in range(6)]
    g = xa[3] - xa[1]
    h = xa[4] - xa[2]
    r3 = g * 2 + h
    r4 = g * -2 + h
    r5 = (xa[5] - xa[3]) + g * -4
    r0 = (xa[0] - xa[2]) * 4 + h
    r2 = (xa[1] - xa[2]) * 4 + (xa[4] - xa[3])
    r1 = (xa[1] + xa[2]) * -4 + (xa[3] + xa[4])
    xt = np.stack([r0, r1, r2, r3, r4, r5])          # [6, 256, H+2, T]
    xt = xt.reshape(6, NCH, P, H + 2, T).transpose(1, 2, 0, 3, 4)
    return np.ascontiguousarray(xt, dtype=np.float16)


def _pack_ww(wcls, wbox):
    """[2][NL, 256, 256, 3, 3] -> [2, NL, 128ip, 2ic, 2oc, 6j, 3k, 128op]."""
    out = np.empty((2, NL, P, NCH, NCH, 6, 3, P), np.float16)
    for s, w in enumerate((wcls, wbox)):
        for l in range(NL):
            t = np.einsum('ja,oika->oikj', G_MAT, w[l].astype(np.float64))
            t = t.reshape(NCH, P, NCH, P, 3, 6).transpose(3, 2, 0, 5, 4, 1)
            out[s, l] = t.astype(np.float16)
    return np.ascontiguousarray(out)


def _pack_pred_wt(w):
    """[n, 256, 3, 3] -> [128ip, 2ic, 6j, 3k, n]."""
    n = w.shape[0]
    t = np.einsum('ja,oika->oikj', G_MAT, w.astype(np.float64))
    t = t.reshape(n, NCH, P, 3, 6).transpose(2, 1, 4, 3, 0)
    return np.ascontiguousarray(t, dtype=np.float16)


def _pack_stem_wp(wcls, wbox):
    w = np.stack([wcls, wbox]).reshape(2, NL, NCH, P, NCH, P, 3, 3)
    w = w.transpose(0, 1, 5, 4, 2, 6, 7, 3)
    return np.ascontiguousarray(w.reshape(2, NL, P, NCH, NCH, 9, P),
                                dtype=np.float16)


def _pack_pred_wp(w):
    n = w.shape[0]
    w = w.reshape(n, NCH, P, 3, 3).transpose(2, 1, 3, 4, 0)
    return np.ascontiguousarray(w.reshape(P, NCH, 9, n), dtype=np.float16)


def kernel(p3, p4, p5, stem_cls_w, stem_cls_b, stem_box_w, stem_box_b,
           pred_cls_w, pred_cls_b, pred_box_w, pred_box_b,
           pred_ctr_w, pred_ctr_b):
    if 'nc' not in _cached:
        _cached['nc'] = _build()
    nc = _cached['nc']

    B = p3.shape[0]
    scw = np.asarray(stem_cls_w, np.float32)
    sbw = np.asarray(stem_box_w, np.float32)
    shared = {
        "sww": _pack_ww(scw, sbw),
        "swp": _pack_stem_wp(scw, sbw),
        "sb": np.ascontiguousarray(
            np.stack([stem_cls_b, stem_box_b]).reshape(2, NL, NCH, P, 1),
            dtype=np.float32),
        "pwct": _pack_pred_wt(np.asarray(pred_cls_w)),
        "pwbt": _pack_pred_wt(
            np.concatenate([pred_box_w, pred_ctr_w], axis=0)),
        "pwc5": _pack_pred_wp(np.asarray(pred_cls_w)),
        "pwb5": _pack_pred_wp(
            np.concatenate([pred_box_w, pred_ctr_w], axis=0)),
        "pbc": np.asarray(pred_cls_b, np.float32).reshape(20, 1),
        "pbb": np.concatenate(
            [pred_box_b, pred_ctr_b]).astype(np.float32).reshape(5, 1),
    }
    in_maps = []
    for b in range(B):
        m = dict(shared)
        m["xt3"] = _wino_xt(np.asarray(p3[b], np.float32))
        m["xt4"] = _wino_xt(np.asarray(p4[b], np.float32))
        m["x5"] = np.pad(
            np.asarray(p5[b], np.float16).reshape(NCH, P, 16, 16),
            ((0, 0), (0, 0), (1, 1), (1, 1)))
        in_maps.append(m)

    res = run_bass_kernel_spmd(nc, in_maps, core_ids=list(range(B)),
                               **_run_opts)
    _last['res'] = res
    out = np.stack([r["out"].T for r in res.results])
    return np.ascontiguousarray(out, dtype=np.float32)


# revision 53
# speedup vs baseline: 1.3933x; 1.0083x over previous
"""FCOS head (nn_FCOS_73787538145418) Trainium2 Bass kernel.

Sharding: data-parallel, one image per NeuronCore (B=8 across 8 cores),
weights replicated. Each core runs the identical SPMD NEFF over its image.

Algorithm: 1D Winograd F(4,3) along W (direct 3-tap conv along H) for the
p3/p4 levels, direct conv for p5. Halves tensor-engine rows for stems and
prediction convs on p3/p4. fp16 operands (1 cyc/row on the PE), fp32 PSUM.
Per conv layer: W-transformed input Xt[j=0..5] (host-computed for the
feature, DVE-computed between layers), 6 PSUM groups m_j accumulated over
(k=3 H-taps x 2 ci chunks), inverse transform y = AT m on DVE/gpsimd,
bias+ReLU on the scalar engine writing a tiled-column spatial layout
[rows, 4, W/4+1] that keeps the next in-transform's reads contiguous.
Output is [25, 5376] channel-major per core; host transposes/stacks.
"""
import sys

if '/opt/trn_rl_repo' not in sys.path:
    sys.path.insert(0, '/opt/trn_rl_repo')

import numpy as np

import concourse.mybir as mybir
from concourse import bacc
import concourse.tile as tile
from concourse.bass_utils import run_bass_kernel_spmd

P = 128
NCH = 2                 # 256 channels = 2 chunks of 128
NL = 4                  # stem depth
NPIX_TOTAL = 5376
F16 = mybir.dt.float16
F32 = mybir.dt.float32
AL = mybir.AluOpType
AF = mybir.ActivationFunctionType

# F(4,3) Winograd (points [0, 1, -1, 2, -2])
G_MAT = np.array([
    [1 / 4, 0, 0], [-1 / 6, -1 / 6, -1 / 6], [-1 / 6, 1 / 6, -1 / 6],
    [1 / 24, 1 / 12, 1 / 6], [1 / 24, -1 / 12, 1 / 6], [0, 0, 1]])

_cached = {}
_run_opts = {}   # extra kwargs for run_bass_kernel_spmd (test harness: trace)
_last = {}       # last BassKernelResults (test harness reads exec_time_ns)


# ---------------------------------------------------------------- views
def _xt_view(flat, off, H, T):
    n = NCH * 6 * (H + 2) * T
    return flat[:, off:off + n].rearrange(
        "p (c j r t) -> p c j r t", c=NCH, j=6, r=H + 2)


def _y_view(flat, off, H, T):
    # tiled-column spatial layout: col = 4*tw + f, tw in [0, T], f in [0, 4)
    n = NCH * (H + 2) * 4 * (T + 1)
    return flat[:, off:off + n].rearrange(
        "p (c r f t) -> p c r f t", c=NCH, r=H + 2, f=4)


def _pad_view(flat, off, H, W):
    n = NCH * (H + 2) * (W + 2)
    return flat[:, off:off + n].rearrange(
        "p (c h w) -> p c h w", c=NCH, h=H + 2, w=W + 2)


def _zero_y_ring(nc, y, H, T):
    nc.vector.memset(y[:, :, 0], 0.0)
    nc.vector.memset(y[:, :, H + 1], 0.0)
    nc.vector.memset(y[:, :, 1:H + 1, 0, 0], 0.0)
    nc.vector.memset(y[:, :, 1:H + 1, 1:4, T], 0.0)


def _zero_ring(nc, v, H, W):
    for c in range(NCH):
        nc.vector.memset(v[:, c, 0, :], 0.0)
        nc.vector.memset(v[:, c, H + 1, :], 0.0)
        nc.vector.memset(v[:, c, 1:H + 1, 0], 0.0)
        nc.vector.memset(v[:, c, 1:H + 1, W + 1], 0.0)


# ------------------------------------------------------- winograd pieces
def _intrans(nc, it_pool, y, xt, H, T, tag, r0=1, r1=None):
    """W-direction F(4,3) input transform: y spatial -> xt[j], rows r0..r1.

    Callers split the row range so the first band's matmuls can start
    after the first chunk instead of the full-image transform."""
    if r1 is None:
        r1 = H + 1
    H = r1 - r0
    xa = [y[:, :, r0:r1, a, 0:T] for a in range(4)]
    xa.append(y[:, :, r0:r1, 0, 1:T + 1])
    xa.append(y[:, :, r0:r1, 1, 1:T + 1])

    def scr(nm):
        return it_pool.tile([P, NCH, H, T], F16, tag="it",
                            name=f"it_{tag}_{nm}")[:]
    V, GP = nc.vector, nc.gpsimd
    g = scr("g"); V.tensor_tensor(g, xa[3], xa[1], AL.subtract)
    h = scr("h"); V.tensor_tensor(h, xa[4], xa[2], AL.subtract)
    # shared double keeps r3/r4 in TensorTensor form (DVE 2x_1p) instead
    # of two 1x TensorScalarPtr ops
    g2 = scr("g2"); V.tensor_tensor(g2, g, g, AL.add)
    V.tensor_tensor(xt[:, :, 3, r0:r1], g2, h, AL.add)
    V.tensor_tensor(xt[:, :, 4, r0:r1], h, g2, AL.subtract)
    m = scr("m"); V.tensor_tensor(m, xa[5], xa[3], AL.subtract)
    V.scalar_tensor_tensor(xt[:, :, 5, r0:r1], g, -4.0, m, AL.mult, AL.add)
    f = scr("f"); V.tensor_tensor(f, xa[0], xa[2], AL.subtract)
    V.scalar_tensor_tensor(xt[:, :, 0, r0:r1], f, 4.0, h, AL.mult, AL.add)
    u = scr("u"); V.tensor_tensor(u, xa[1], xa[2], AL.subtract)
    v = scr("v"); V.tensor_tensor(v, xa[4], xa[3], AL.subtract)
    V.scalar_tensor_tensor(xt[:, :, 2, r0:r1], u, 4.0, v, AL.mult, AL.add)
    p_ = scr("p"); V.tensor_tensor(p_, xa[1], xa[2], AL.add)
    q = scr("q"); V.tensor_tensor(q, xa[3], xa[4], AL.add)
    V.scalar_tensor_tensor(xt[:, :, 1, r0:r1], p_, -4.0, q, AL.mult, AL.add)


def _wino_unit(nc, psum_pool, ot_pool, ya_pool, lhsT_fn, xt, r0, R, T,
               nlo, nhi, tag):
    """One band: 6 PSUM groups (3k x 2ci matmuls each) + inverse transform.

    lhsT_fn(c, j, k) -> weight AP [K=128, M]; output written to partitions
    nlo:nhi of psum/scr tiles. Returns yact tile view [nlo:nhi, R, 4, T]."""
    def scr(nm):
        t = ot_pool.tile([P, R, T], F16, tag="ot", name=f"ot_{tag}_{nm}")
        return t[nlo:nhi]

    cs = []
    def mm(j):
        # 6 accumulating matmuls into one PSUM group, then a scalar-engine
        # copy to fp16 SBUF (PSUM allows only one engine-instruction input;
        # the copy also releases the PSUM bank early)
        ps = psum_pool.tile([P, R, T], F32, tag="ps", name=f"ps_{tag}_{j}")
        kk = 0
        for c in range(NCH):
            for k in range(3):
                nc.tensor.matmul(ps[nlo:nhi], lhsT_fn(c, j, k),
                                 xt[:, c, j, r0 + k:r0 + k + R, :],
                                 start=(kk == 0), stop=(kk == 5))
                kk += 1
        cj = scr(f"c{j}")
        nc.scalar.activation(cj, ps[nlo:nhi], AF.Copy)
        cs.append(cj)

    ya = ya_pool.tile([P, R, 4, T], F16, tag="ya", name=f"ya_{tag}")
    V, GP = nc.vector, nc.gpsimd

    mm(0); mm(1); mm(2)
    s = scr("s"); V.tensor_tensor(s, cs[1], cs[2], AL.add)
    d = scr("d"); V.tensor_tensor(d, cs[1], cs[2], AL.subtract)
    mm(3); mm(4)
    # DVE TensorTensor runs in 2x_1p mode (~4x cheaper than gpsimd);
    # gpsimd keeps only S/D so y1/y2 can start while DVE finishes u/y0
    u = scr("u"); V.tensor_tensor(u, cs[0], s, AL.add)
    S = scr("S"); V.tensor_tensor(S, cs[3], cs[4], AL.add)
    D = scr("D"); V.tensor_tensor(D, cs[3], cs[4], AL.subtract)
    V.scalar_tensor_tensor(ya[nlo:nhi, :, 1, :], D, 2.0, d, AL.mult, AL.add)
    V.scalar_tensor_tensor(ya[nlo:nhi, :, 2, :], S, 4.0, s, AL.mult, AL.add)
    mm(5)
    V.tensor_tensor(ya[nlo:nhi, :, 0, :], u, S, AL.add)
    v3 = scr("v3")
    V.scalar_tensor_tensor(v3, D, 8.0, d, AL.mult, AL.add)
    V.tensor_tensor(ya[nlo:nhi, :, 3, :], v3, cs[5], AL.add)
    return ya


def _wino_layer(nc, pools, wt, xt, ydst, bias_ap, H, T, tag, plain=False):
    """Full 256->256 W-winograd conv + bias + relu.

    plain=False: ydst is the tiled-column layout (feeds next in-transform).
    plain=True: ydst is a plain padded [c, H+2, W+2] view (feeds direct
    prediction convs); the activation collapses to one instruction."""
    psum_pool, ot_pool, ya_pool = pools
    bands = [(0, 32), (32, 32)] if H == 64 else [(0, H)]
    W = 4 * T
    for bi, (r0, R) in enumerate(bands):
        for o in range(NCH):
            ya = _wino_unit(nc, psum_pool, ot_pool, ya_pool,
                            lambda c, j, k: wt[:, c, o, j, k, :],
                            xt, r0, R, T, 0, P, f"{tag}{bi}{o}")
            rows = slice(r0 + 1, r0 + 1 + R)
            if plain:
                dv = ydst[:, o, rows, 1:W + 1].rearrange(
                    "p r (t f) -> p r f t", f=4)
                nc.scalar.activation(dv, ya[:], AF.Relu, bias=bias_ap[:, o])
            else:
                nc.scalar.activation(ydst[:, o, rows, 1:4, 0:T],
                                     ya[:, :, 0:3, :], AF.Relu,
                                     bias=bias_ap[:, o])
                nc.scalar.activation(ydst[:, o, rows, 0, 1:T + 1],
                                     ya[:, :, 3, :], AF.Relu,
                                     bias=bias_ap[:, o])


def _wino_preds(nc, pools, stage_pool, pwct, pwbt, pbc_t, pbb_t,
                xtc, xtb, out_d, H, T, pix_base, tag):
    """cls(20ch) + box/ctr(5ch) W-winograd pred convs + bias (no relu)."""
    psum_pool, ot_pool, ya_pool = pools
    bands = [(0, 32), (32, 32)] if H == 64 else [(0, H)]
    for bi, (r0, R) in enumerate(bands):
        n = R * T * 4
        c0 = pix_base + r0 * T * 4
        for hi, (pw, nout, xt, pb, olo) in enumerate(
                ((pwct, 20, xtc, pbc_t, 0), (pwbt, 5, xtb, pbb_t, 20))):
            ya = _wino_unit(nc, psum_pool, ot_pool, ya_pool,
                            lambda c, j, k: pw[:, c, j, k, :],
                            xt, r0, R, T, 0, nout, f"{tag}p{bi}{hi}")
            st = stage_pool.tile([32, R, T, 4], F32, tag="st",
                                 name=f"st_{tag}{bi}{hi}")
            stv = st.rearrange("p r t f -> p r f t")
            nc.scalar.activation(stv[0:nout], ya[0:nout], AF.Identity,
                                 bias=pb[0:nout])
            nc.sync.dma_start(
                out_d[olo:olo + nout, c0:c0 + n],
                st[0:nout].rearrange("p r t f -> p (r t f)"))


# ------------------------------------------------------- p5 direct path
def _conv_layer(nc, psum_pool, wt, src, dst, bias_ap, H, W, R, tag):
    n_tiles = H // R
    for o in range(NCH):
        pss = [psum_pool.tile([P, R, W], F32, tag="ps",
                              name=f"ps_{tag}_{o}_{it}")
               for it in range(n_tiles)]
        k = 0
        for c in range(NCH):
            for ky in range(3):
                for kx in range(3):
                    lhsT = wt[:, c, o, ky * 3 + kx, :]
                    for it in range(n_tiles):
                        r0 = it * R
                        rhs = src[:, c, r0 + ky:r0 + ky + R, kx:kx + W]
                        nc.tensor.matmul(pss[it][:], lhsT, rhs,
                                         start=(k == 0), stop=(k == 17))
                    k += 1
        for it in range(n_tiles):
            r0 = it * R
            nc.scalar.activation(dst[:, o, r0 + 1:r0 + 1 + R, 1:W + 1],
                                 pss[it][:], AF.Relu, bias=bias_ap[:, o])


def _preds5_head(nc, psum_pool, stage_pool, pw, pb_t, tower, out_d,
                 H, W, R, pix_base, olo, nout, tag, it):
    """One R-row band of ONE direct prediction head (cls or box/ctr)."""
    r0 = it * R
    ps = psum_pool.tile([P, R, W], F32, tag="ps", name=f"ph_{tag}_{it}")
    k = 0
    for c in range(NCH):
        for ky in range(3):
            for kx in range(3):
                t = ky * 3 + kx
                nc.tensor.matmul(ps[0:nout], pw[:, c, t, :],
                                 tower[:, c, r0 + ky:r0 + ky + R, kx:kx + W],
                                 start=(k == 0), stop=(k == 17))
                k += 1
    st = stage_pool.tile([32, R, W, 1], F32, tag="st", name=f"sh_{tag}_{it}")
    sf = st.rearrange("p r w o -> p (r w o)")
    nc.scalar.activation(sf[0:nout], ps[0:nout].rearrange("p r w -> p (r w)"),
                         AF.Identity, bias=pb_t[0:nout])
    c0 = pix_base + r0 * W
    nc.sync.dma_start(out_d[olo:olo + nout, c0:c0 + R * W], sf[0:nout])


def _preds5(nc, psum_pool, stage_pool, pwc, pwb, pbc_t, pbb_t,
            cls_tower, box_tower, out_d, H, W, R, pix_base, tag):
    n_tiles = H // R
    for it in range(n_tiles):
        r0 = it * R
        ps1 = psum_pool.tile([P, R, W], F32, tag="ps", name=f"pc_{tag}_{it}")
        ps2 = psum_pool.tile([P, R, W], F32, tag="ps", name=f"pb_{tag}_{it}")
        k = 0
        for c in range(NCH):
            for ky in range(3):
                for kx in range(3):
                    t = ky * 3 + kx
                    rc = cls_tower[:, c, r0 + ky:r0 + ky + R, kx:kx + W]
                    rb = box_tower[:, c, r0 + ky:r0 + ky + R, kx:kx + W]
                    nc.tensor.matmul(ps1[0:20], pwc[:, c, t, :], rc,
                                     start=(k == 0), stop=(k == 17))
                    nc.tensor.matmul(ps2[0:5], pwb[:, c, t, :], rb,
                                     start=(k == 0), stop=(k == 17))
                    k += 1
        st = stage_pool.tile([32, R, W, 1], F32, tag="st", name=f"s5_{tag}_{it}")
        sf = st.rearrange("p r w o -> p (r w o)")
        nc.scalar.activation(sf[0:20], ps1[0:20].rearrange("p r w -> p (r w)"),
                             AF.Identity, bias=pbc_t[0:20])
        st2 = stage_pool.tile([32, R, W, 1], F32, tag="st", name=f"s6_{tag}_{it}")
        sf2 = st2.rearrange("p r w o -> p (r w o)")
        nc.scalar.activation(sf2[0:5], ps2[0:5].rearrange("p r w -> p (r w)"),
                             AF.Identity, bias=pbb_t[0:5])
        c0 = pix_base + r0 * W
        nc.sync.dma_start(out_d[0:20, c0:c0 + R * W], sf[0:20])
        nc.sync.dma_start(out_d[20:25, c0:c0 + R * W], sf2[0:5])


# ------------------------------------------------------------ weight DMA
def _load_ww(nc, wtw_pool, sww_d, s, l, tag, fine=False):
    wt = wtw_pool.tile([P, NCH, NCH, 6, 3, P], F16, tag="ww",
                       name=f"ww_{tag}_{s}_{l}")
    if fine:
        # per-(o, j, c) splits so the first matmuls' deps clear quickly
        for o in range(NCH):
            for j in range(6):
                for c in range(NCH):
                    nc.sync.dma_start(wt[:, c, o, j], sww_d[s, l, :, c, o, j])
    else:
        for c in range(NCH):
            for o in range(NCH):
                nc.sync.dma_start(wt[:, c, o], sww_d[s, l, :, c, o])
    return wt


def _load_wp(nc, wtp_pool, swp_d, s, l, tag):
    wt = wtp_pool.tile([P, NCH, NCH, 9, P], F16, tag="wp",
                       name=f"wp_{tag}_{s}_{l}")
    for c in range(NCH):
        for o in range(NCH):
            nc.sync.dma_start(wt[:, c, o], swp_d[s, l, :, c, o])
    return wt


# ------------------------------------------------------------------ build
def _build():
    nc = bacc.Bacc("TRN2", target_bir_lowering=False, debug=False,
                   num_devices=8)

    # p3 Xt0 (host-transformed), shipped once per chain buffer
    xt3_d = nc.dram_tensor("xt3", (NCH, P, 6, 66, 16), F16,
                           kind="ExternalInput")
    xt4_d = nc.dram_tensor("xt4", (NCH, P, 6, 34, 8), F16,
                           kind="ExternalInput")
    x5_d = nc.dram_tensor("x5", (NCH, P, 18, 18), F16, kind="ExternalInput")
    sww_d = nc.dram_tensor("sww", (2, NL, P, NCH, NCH, 6, 3, P), F16,
                           kind="ExternalInput")
    swp_d = nc.dram_tensor("swp", (2, NL, P, NCH, NCH, 9, P), F16,
                           kind="ExternalInput")
    sb_d = nc.dram_tensor("sb", (2, NL, NCH, P, 1), F32, kind="ExternalInput")
    pwct_d = nc.dram_tensor("pwct", (P, NCH, 6, 3, 20), F16,
                            kind="ExternalInput")
    pwbt_d = nc.dram_tensor("pwbt", (P, NCH, 6, 3, 5), F16,
                            kind="ExternalInput")
    pwc5_d = nc.dram_tensor("pwc5", (P, NCH, 9, 20), F16,
                            kind="ExternalInput")
    pwb5_d = nc.dram_tensor("pwb5", (P, NCH, 9, 5), F16,
                            kind="ExternalInput")
    pbc_d = nc.dram_tensor("pbc", (20, 1), F32, kind="ExternalInput")
    pbb_d = nc.dram_tensor("pbb", (5, 1), F32, kind="ExternalInput")
    out_d = nc.dram_tensor("out", (25, NPIX_TOTAL), F32, kind="ExternalOutput")

    NXT3 = NCH * 6 * 66 * 16      # 12672
    NY3 = NCH * 66 * 68           # 8976
    NXT4 = NCH * 6 * 34 * 8       # 3264
    NY4 = NCH * 34 * 36           # 2448
    N5 = NCH * 18 * 18            # 648

    with tile.TileContext(nc) as tc:
        with (
            tc.tile_pool(name="resident", bufs=1) as res_pool,
            tc.tile_pool(name="wtw", bufs=2) as wtw_pool,
            tc.tile_pool(name="wtp", bufs=1) as wtp_pool,
            tc.tile_pool(name="psum", bufs=8, space="PSUM") as psum_pool,
            tc.tile_pool(name="stage", bufs=2) as stage_pool,
            tc.tile_pool(name="it", bufs=5) as it_pool,
            tc.tile_pool(name="ot", bufs=18) as ot_pool,
            tc.tile_pool(name="ya", bufs=4) as ya_pool,
        ):
            xtc_f = res_pool.tile([P, NXT3], F16, name="xtc")
            xtb_f = res_pool.tile([P, NXT3], F16, name="xtb")
            yc_f = res_pool.tile([P, NY3], F16, name="yc")
            yb_f = res_pool.tile([P, NY3], F16, name="yb")

            sbias = res_pool.tile([P, 2, NL, NCH, 1], F32, name="sbias")
            pwct = res_pool.tile([P, NCH, 6, 3, 20], F16, name="pwct")
            pwbt = res_pool.tile([P, NCH, 6, 3, 5], F16, name="pwbt")
            pwc5 = res_pool.tile([P, NCH, 9, 20], F16, name="pwc5")
            pwb5 = res_pool.tile([P, NCH, 9, 5], F16, name="pwb5")
            pbc_t = res_pool.tile([32, 1], F32, name="pbct")
            pbb_t = res_pool.tile([32, 1], F32, name="pbbt")

            xtc = _xt_view(xtc_f, 0, 64, 16)
            xtb = _xt_view(xtb_f, 0, 64, 16)
            yc = _y_view(yc_f, 0, 64, 16)
            yb = _y_view(yb_f, 0, 64, 16)
            # pass-b carvings
            xtc4 = _xt_view(yc_f, 0, 32, 8)
            yc4 = _y_view(yc_f, NXT4, 32, 8)
            xtb4 = _xt_view(yb_f, 0, 32, 8)
            yb4 = _y_view(yb_f, NXT4, 32, 8)
            # p5 runs inside pass A (its direct matmuls fill tensor idle
            # and cost no vector-engine work) -> static buffers
            x5_f = res_pool.tile([P, 4 * N5], F16, name="x5f")
            v5 = [_pad_view(x5_f, i * N5, 16, 16) for i in range(4)]

            pools = (psum_pool, ot_pool, ya_pool)

            # ---- startup DMAs: first-layer weights + cls Xt0, then rest
            # the very first matmuls' Xt chunks lead the sync queue
            for c in range(NCH):
                nc.sync.dma_start(xtc[:, c, 0, 0:33], xt3_d[c, :, 0, 0:33])
            ww00 = _load_ww(nc, wtw_pool, sww_d, 0, 0, "a", fine=True)
            # remaining Xt0 chunks ride the gpsimd SWDGE queue, parallel to
            # the startup-critical weight DMAs on the sync queue
            for j in range(6):
                for c in range(NCH):
                    if j > 0:
                        nc.gpsimd.dma_start(xtc[:, c, j, 0:33],
                                            xt3_d[c, :, j, 0:33])
                    nc.gpsimd.dma_start(xtc[:, c, j, 33:66],
                                        xt3_d[c, :, j, 33:66])
            for c in range(NCH):
                nc.sync.dma_start(xtb[:, c], xt3_d[c])
            nc.gpsimd.dma_start(
                sbias[:],
                sb_d[:].rearrange("s l a p o -> p (s l a o)")
                       .rearrange("p (s l a o) -> p s l a o",
                                  s=2, l=NL, a=NCH))
            nc.gpsimd.dma_start(pwct[:], pwct_d[:])
            nc.gpsimd.dma_start(pwbt[:], pwbt_d[:])
            nc.gpsimd.dma_start(pwc5[:], pwc5_d[:])
            nc.gpsimd.dma_start(pwb5[:], pwb5_d[:])
            nc.gpsimd.dma_start(pbc_t[0:20], pbc_d[:])
            nc.gpsimd.dma_start(pbb_t[0:5], pbb_d[:])
            for c in range(NCH):
                nc.gpsimd.dma_start(v5[0][:, c], x5_d[c])
            _zero_y_ring(nc, yc, 64, 16)
            _zero_y_ring(nc, yb, 64, 16)
            for i in range(1, 4):
                _zero_ring(nc, v5[i], 16, 16)

            # ---- pass A: p3 winograd + p5 direct, chains interleaved
            # p5 rotation: cls v0->v1->v3->v1->v3 (tower v3);
            #              box v0->v2->v0->v2->v0 (tower v0)
            cls_chain = [(0, 1), (1, 3), (3, 1), (1, 3)]
            box_chain = [(0, 2), (2, 0), (0, 2), (2, 0)]
            ww = {(0, 0): ww00}

            def get_ww(s, l, tag):
                if (s, l) not in ww:
                    ww[(s, l)] = _load_ww(nc, wtw_pool, sww_d, s, l, tag)
                return ww[(s, l)]

            ycp = _pad_view(yc_f, 0, 64, 64)
            ybp = _pad_view(yb_f, 0, 64, 64)
            y4cp = _pad_view(yc_f, NXT4, 32, 32)
            y4bp = _pad_view(yb_f, NXT4, 32, 32)

            for l in range(NL):
                for s, xt, y, chain in ((0, xtc, yc, cls_chain),
                                        (1, xtb, yb, box_chain)):
                    wt = get_ww(s, l, "a")
                    wp = _load_wp(nc, wtp_pool, swp_d, s, l, "a")
                    if l > 0:
                        _intrans(nc, it_pool, y, xt, 64, 16, f"a{s}{l}A",
                                 1, 34)
                        _intrans(nc, it_pool, y, xt, 64, 16, f"a{s}{l}B",
                                 34, 65)
                    if l == NL - 1:
                        # towers in plain layout: preds run as direct convs
                        yp = ycp if s == 0 else ybp
                        _zero_ring(nc, yp, 64, 64)
                        _wino_layer(nc, pools, wt, xt, yp, sbias[:, s, l],
                                    64, 16, f"a{s}{l}", plain=True)
                    else:
                        _wino_layer(nc, pools, wt, xt, y, sbias[:, s, l],
                                    64, 16, f"a{s}{l}")
                    si, di = chain[l]
                    _conv_layer(nc, psum_pool, wp, v5[si], v5[di],
                                sbias[:, s, l], 16, 16, 16, f"a5{s}{l}")
                    if l == NL - 1 and s == 0:
                        # cls-head pred bands run during the box chain's
                        # final layer (its transforms own DVE/gpsimd then)
                        for it in range(8):
                            _preds5_head(nc, psum_pool, stage_pool, pwc5,
                                         pbc_t, ycp, out_d, 64, 64, 8, 0,
                                         0, 20, "a3c", it)
                    # prefetch next (s, l) winograd weights
                    nl_s, nl_l = (1, l) if s == 0 else (0, l + 1)
                    if nl_l < NL:
                        get_ww(nl_s, nl_l, "a")
            for it in range(8):
                _preds5_head(nc, psum_pool, stage_pool, pwb5, pbb_t, ybp,
                             out_d, 64, 64, 8, 0, 20, 5, "a3b", it)
            _preds5(nc, psum_pool, stage_pool, pwc5, pwb5, pbc_t, pbb_t,
                    v5[3], v5[0], out_d, 16, 16, 16, 5120, "a5")
            # p4 staging into regions freed once the p3 preds are done
            for c in range(NCH):
                nc.sync.dma_start(xtc4[:, c], xt4_d[c])
                nc.sync.dma_start(xtb4[:, c], xt4_d[c])
            _zero_y_ring(nc, yc4, 32, 8)
            _zero_y_ring(nc, yb4, 32, 8)

            # ---- pass B: p4 winograd stems + direct preds
            for l in range(NL):
                for s, xt4, y4 in ((0, xtc4, yc4), (1, xtb4, yb4)):
                    # reload winograd weights for pass b (pool rotation)
                    wt = _load_ww(nc, wtw_pool, sww_d, s, l, "b")
                    if l > 0:
                        _intrans(nc, it_pool, y4, xt4, 32, 8, f"b{s}{l}")
                    if l == NL - 1:
                        yp = y4cp if s == 0 else y4bp
                        _zero_ring(nc, yp, 32, 32)
                        _wino_layer(nc, pools, wt, xt4, yp, sbias[:, s, l],
                                    32, 8, f"b{s}{l}", plain=True)
                    else:
                        _wino_layer(nc, pools, wt, xt4, y4, sbias[:, s, l],
                                    32, 8, f"b{s}{l}")
            _preds5(nc, psum_pool, stage_pool, pwc5, pwb5, pbc_t, pbb_t,
                    y4cp, y4bp, out_d, 32, 32, 16, 4096, "b3")

    nc.compile()
    return nc


# ------------------------------------------------------------- host side
def _wino_xt(x):
    """x [256, H, W] fp32 -> Xt [2, 128, 6, H+2, W//4] fp16."""
    Cc, H, W = x.shape
    T = W // 4
    xp = np.pad(x, ((0, 0), (1, 1), (1, 3))).astype(np.float32)
    xa = [xp[:, :, a:a + 4 * T:4] for a in range(6)]
    g = xa[3] - xa[1]
    h = xa[4] - xa[2]
    r3 = g * 2 + h
    r4 = g * -2 + h
    r5 = (xa[5] - xa[3]) + g * -4
    r0 = (xa[0] - xa[2]) * 4 + h
    r2 = (xa[1] - xa[2]) * 4 + (xa[4] - xa[3])
    r1 = (xa[1] + xa[2]) * -4 + (xa[3] + xa[4])
    xt = np.stack([r0, r1, r2, r3, r4, r5])          # [6, 256, H+2, T]
    xt = xt.reshape(6, NCH, P, H + 2, T).transpose(1, 2, 0, 3, 4)
    return np.ascontiguousarray(xt, dtype=np.float16)


def _pack_ww(wcls, wbox):
    """[2][NL, 256, 256, 3, 3] -> [2, NL, 128ip, 2ic, 2oc, 6j, 3k, 128op]."""
    out = np.empty((2, NL, P, NCH, NCH, 6, 3, P), np.float16)
    for s, w in enumerate((wcls, wbox)):
        for l in range(NL):
            t = np.einsum('ja,oika->oikj', G_MAT, w[l].astype(np.float64))
            t = t.reshape(NCH, P, NCH, P, 3, 6).transpose(3, 2, 0, 5, 4, 1)
            out[s, l] = t.astype(np.float16)
    return np.ascontiguousarray(out)


def _pack_pred_wt(w):
    """[n, 256, 3, 3] -> [128ip, 2ic, 6j, 3k, n]."""
    n = w.shape[0]
    t = np.einsum('ja,oika->oikj', G_MAT, w.astype(np.float64))
    t = t.reshape(n, NCH, P, 3, 6).transpose(2, 1, 4, 3, 0)
    return np.ascontiguousarray(t, dtype=np.float16)


def _pack_stem_wp(wcls, wbox):
    w = np.stack([wcls, wbox]).reshape(2, NL, NCH, P, NCH, P, 3, 3)
    w = w.transpose(0, 1, 5, 4, 2, 6, 7, 3)
    return np.ascontiguousarray(w.reshape(2, NL, P, NCH, NCH, 9, P),
                                dtype=np.float16)


def _pack_pred_wp(w):
    n = w.shape[0]
    w = w.reshape(n, NCH, P, 3, 3).transpose(2, 1, 3, 4, 0)
    return np.ascontiguousarray(w.reshape(P, NCH, 9, n), dtype=np.float16)


def kernel(p3, p4, p5, stem_cls_w, stem_cls_b, stem_box_w, stem_box_b,
           pred_cls_w, pred_cls_b, pred_box_w, pred_box_b,
           pred_ctr_w, pred_ctr_b):
    if 'nc' not in _cached:
        _cached['nc'] = _build()
    nc = _cached['nc']

    B = p3.shape[0]
    scw = np.asarray(stem_cls_w, np.float32)
    sbw = np.asarray(stem_box_w, np.float32)
    shared = {
        "sww": _pack_ww(scw, sbw),
        "swp": _pack_stem_wp(scw, sbw),
        "sb": np.ascontiguousarray(
            np.stack([stem_cls_b, stem_box_b]).reshape(2, NL, NCH, P, 1),
            dtype=np.float32),
        "pwct": _pack_pred_wt(np.asarray(pred_cls_w)),
        "pwbt": _pack_pred_wt(
            np.concatenate([pred_box_w, pred_ctr_w], axis=0)),
        "pwc5": _pack_pred_wp(np.asarray(pred_cls_w)),
        "pwb5": _pack_pred_wp(
            np.concatenate([pred_box_w, pred_ctr_w], axis=0)),
        "pbc": np.asarray(pred_cls_b, np.float32).reshape(20, 1),
        "pbb": np.concatenate(
            [pred_box_b, pred_ctr_b]).astype(np.float32).reshape(5, 1),
    }
    in_maps = []
    for b in range(B):
        m = dict(shared)
        m["xt3"] = _wino_xt(np.asarray(p3[b], np.float32))
        m["xt4"] = _wino_xt(np.asarray(p4[b], np.float32))
        m["x5"] = np.pad(
            np.asarray(p5[b], np.float16).reshape(NCH, P, 16, 16),
            ((0, 0), (0, 0), (1, 1), (1, 1)))
        in_maps.append(m)

    res = run_bass_kernel_spmd(nc, in_maps, core_ids=list(range(B)),
                               **_run_opts)
    _last['res'] = res
    out = np.stack([r["out"].T for r in res.results])
    return np.ascontiguousarray(out, dtype=np.float32)


# revision 54
# speedup vs baseline: 1.4039x; 1.0076x over previous
"""FCOS head (nn_FCOS_73787538145418) Trainium2 Bass kernel.

Sharding: data-parallel, one image per NeuronCore (B=8 across 8 cores),
weights replicated. Each core runs the identical SPMD NEFF over its image.

Algorithm: 1D Winograd F(4,3) along W (direct 3-tap conv along H) for the
p3/p4 levels, direct conv for p5. Halves tensor-engine rows for stems and
prediction convs on p3/p4. fp16 operands (1 cyc/row on the PE), fp32 PSUM.
Per conv layer: W-transformed input Xt[j=0..5] (host-computed for the
feature, DVE-computed between layers), 6 PSUM groups m_j accumulated over
(k=3 H-taps x 2 ci chunks), inverse transform y = AT m on DVE/gpsimd,
bias+ReLU on the scalar engine writing a tiled-column spatial layout
[rows, 4, W/4+1] that keeps the next in-transform's reads contiguous.
Output is [25, 5376] channel-major per core; host transposes/stacks.
"""
import sys

if '/opt/trn_rl_repo' not in sys.path:
    sys.path.insert(0, '/opt/trn_rl_repo')

import numpy as np

import concourse.mybir as mybir
from concourse import bacc
import concourse.tile as tile
from concourse.bass_utils import run_bass_kernel_spmd

P = 128
NCH = 2                 # 256 channels = 2 chunks of 128
NL = 4                  # stem depth
NPIX_TOTAL = 5376
F16 = mybir.dt.float16
F32 = mybir.dt.float32
AL = mybir.AluOpType
AF = mybir.ActivationFunctionType

# F(4,3) Winograd (points [0, 1, -1, 2, -2])
G_MAT = np.array([
    [1 / 4, 0, 0], [-1 / 6, -1 / 6, -1 / 6], [-1 / 6, 1 / 6, -1 / 6],
    [1 / 24, 1 / 12, 1 / 6], [1 / 24, -1 / 12, 1 / 6], [0, 0, 1]])

_cached = {}
_run_opts = {}   # extra kwargs for run_bass_kernel_spmd (test harness: trace)
_last = {}       # last BassKernelResults (test harness reads exec_time_ns)


# ---------------------------------------------------------------- views
def _xt_view(flat, off, H, T):
    n = NCH * 6 * (H + 2) * T
    return flat[:, off:off + n].rearrange(
        "p (c j r t) -> p c j r t", c=NCH, j=6, r=H + 2)


def _y_view(flat, off, H, T):
    # tiled-column spatial layout: col = 4*tw + f, tw in [0, T], f in [0, 4)
    n = NCH * (H + 2) * 4 * (T + 1)
    return flat[:, off:off + n].rearrange(
        "p (c r f t) -> p c r f t", c=NCH, r=H + 2, f=4)


def _pad_view(flat, off, H, W):
    n = NCH * (H + 2) * (W + 2)
    return flat[:, off:off + n].rearrange(
        "p (c h w) -> p c h w", c=NCH, h=H + 2, w=W + 2)


def _zero_y_ring(nc, y, H, T):
    nc.vector.memset(y[:, :, 0], 0.0)
    nc.vector.memset(y[:, :, H + 1], 0.0)
    nc.vector.memset(y[:, :, 1:H + 1, 0, 0], 0.0)
    nc.vector.memset(y[:, :, 1:H + 1, 1:4, T], 0.0)


def _zero_ring(nc, v, H, W):
    for c in range(NCH):
        nc.vector.memset(v[:, c, 0, :], 0.0)
        nc.vector.memset(v[:, c, H + 1, :], 0.0)
        nc.vector.memset(v[:, c, 1:H + 1, 0], 0.0)
        nc.vector.memset(v[:, c, 1:H + 1, W + 1], 0.0)


# ------------------------------------------------------- winograd pieces
def _intrans(nc, it_pool, y, xt, H, T, tag, r0=1, r1=None):
    """W-direction F(4,3) input transform: y spatial -> xt[j], rows r0..r1.

    Callers split the row range so the first band's matmuls can start
    after the first chunk instead of the full-image transform."""
    if r1 is None:
        r1 = H + 1
    H = r1 - r0
    xa = [y[:, :, r0:r1, a, 0:T] for a in range(4)]
    xa.append(y[:, :, r0:r1, 0, 1:T + 1])
    xa.append(y[:, :, r0:r1, 1, 1:T + 1])

    def scr(nm):
        return it_pool.tile([P, NCH, H, T], F16, tag="it",
                            name=f"it_{tag}_{nm}")[:]
    V, GP = nc.vector, nc.gpsimd
    g = scr("g"); V.tensor_tensor(g, xa[3], xa[1], AL.subtract)
    h = scr("h"); V.tensor_tensor(h, xa[4], xa[2], AL.subtract)
    # shared double keeps r3/r4 in TensorTensor form (DVE 2x_1p) instead
    # of two 1x TensorScalarPtr ops
    g2 = scr("g2"); V.tensor_tensor(g2, g, g, AL.add)
    V.tensor_tensor(xt[:, :, 3, r0:r1], g2, h, AL.add)
    V.tensor_tensor(xt[:, :, 4, r0:r1], h, g2, AL.subtract)
    m = scr("m"); V.tensor_tensor(m, xa[5], xa[3], AL.subtract)
    V.scalar_tensor_tensor(xt[:, :, 5, r0:r1], g, -4.0, m, AL.mult, AL.add)
    f = scr("f"); V.tensor_tensor(f, xa[0], xa[2], AL.subtract)
    V.scalar_tensor_tensor(xt[:, :, 0, r0:r1], f, 4.0, h, AL.mult, AL.add)
    u = scr("u"); V.tensor_tensor(u, xa[1], xa[2], AL.subtract)
    v = scr("v"); V.tensor_tensor(v, xa[4], xa[3], AL.subtract)
    V.scalar_tensor_tensor(xt[:, :, 2, r0:r1], u, 4.0, v, AL.mult, AL.add)
    p_ = scr("p"); V.tensor_tensor(p_, xa[1], xa[2], AL.add)
    q = scr("q"); V.tensor_tensor(q, xa[3], xa[4], AL.add)
    V.scalar_tensor_tensor(xt[:, :, 1, r0:r1], p_, -4.0, q, AL.mult, AL.add)


def _wino_unit(nc, psum_pool, ot_pool, ya_pool, lhsT_fn, xt, r0, R, T,
               nlo, nhi, tag):
    """One band: 6 PSUM groups (3k x 2ci matmuls each) + inverse transform.

    lhsT_fn(c, j, k) -> weight AP [K=128, M]; output written to partitions
    nlo:nhi of psum/scr tiles. Returns yact tile view [nlo:nhi, R, 4, T]."""
    def scr(nm):
        t = ot_pool.tile([P, R, T], F16, tag="ot", name=f"ot_{tag}_{nm}")
        return t[nlo:nhi]

    cs = []
    def mm(j):
        # 6 accumulating matmuls into one PSUM group, then a scalar-engine
        # copy to fp16 SBUF (PSUM allows only one engine-instruction input;
        # the copy also releases the PSUM bank early)
        ps = psum_pool.tile([P, R, T], F32, tag="ps", name=f"ps_{tag}_{j}")
        kk = 0
        for c in range(NCH):
            for k in range(3):
                nc.tensor.matmul(ps[nlo:nhi], lhsT_fn(c, j, k),
                                 xt[:, c, j, r0 + k:r0 + k + R, :],
                                 start=(kk == 0), stop=(kk == 5))
                kk += 1
        cj = scr(f"c{j}")
        nc.scalar.activation(cj, ps[nlo:nhi], AF.Copy)
        cs.append(cj)

    ya = ya_pool.tile([P, R, 4, T], F16, tag="ya", name=f"ya_{tag}")
    V, GP = nc.vector, nc.gpsimd

    mm(0); mm(1); mm(2)
    s = scr("s"); V.tensor_tensor(s, cs[1], cs[2], AL.add)
    d = scr("d"); V.tensor_tensor(d, cs[1], cs[2], AL.subtract)
    mm(3); mm(4)
    # DVE TensorTensor runs in 2x_1p mode (~4x cheaper than gpsimd);
    # gpsimd keeps only S/D so y1/y2 can start while DVE finishes u/y0
    u = scr("u"); V.tensor_tensor(u, cs[0], s, AL.add)
    S = scr("S"); V.tensor_tensor(S, cs[3], cs[4], AL.add)
    D = scr("D"); V.tensor_tensor(D, cs[3], cs[4], AL.subtract)
    V.scalar_tensor_tensor(ya[nlo:nhi, :, 1, :], D, 2.0, d, AL.mult, AL.add)
    V.scalar_tensor_tensor(ya[nlo:nhi, :, 2, :], S, 4.0, s, AL.mult, AL.add)
    mm(5)
    V.tensor_tensor(ya[nlo:nhi, :, 0, :], u, S, AL.add)
    v3 = scr("v3")
    V.scalar_tensor_tensor(v3, D, 8.0, d, AL.mult, AL.add)
    V.tensor_tensor(ya[nlo:nhi, :, 3, :], v3, cs[5], AL.add)
    return ya


def _wino_layer(nc, pools, wt, xt, ydst, bias_ap, H, T, tag, plain=False):
    """Full 256->256 W-winograd conv + bias + relu.

    plain=False: ydst is the tiled-column layout (feeds next in-transform).
    plain=True: ydst is a plain padded [c, H+2, W+2] view (feeds direct
    prediction convs); the activation collapses to one instruction."""
    psum_pool, ot_pool, ya_pool = pools
    bands = [(0, 32), (32, 32)] if H == 64 else [(0, H)]
    W = 4 * T
    for bi, (r0, R) in enumerate(bands):
        for o in range(NCH):
            ya = _wino_unit(nc, psum_pool, ot_pool, ya_pool,
                            lambda c, j, k: wt[:, c, o, j, k, :],
                            xt, r0, R, T, 0, P, f"{tag}{bi}{o}")
            rows = slice(r0 + 1, r0 + 1 + R)
            if plain:
                dv = ydst[:, o, rows, 1:W + 1].rearrange(
                    "p r (t f) -> p r f t", f=4)
                nc.scalar.activation(dv, ya[:], AF.Relu, bias=bias_ap[:, o])
            else:
                nc.scalar.activation(ydst[:, o, rows, 1:4, 0:T],
                                     ya[:, :, 0:3, :], AF.Relu,
                                     bias=bias_ap[:, o])
                nc.scalar.activation(ydst[:, o, rows, 0, 1:T + 1],
                                     ya[:, :, 3, :], AF.Relu,
                                     bias=bias_ap[:, o])


def _wino_preds(nc, pools, stage_pool, pwct, pwbt, pbc_t, pbb_t,
                xtc, xtb, out_d, H, T, pix_base, tag):
    """cls(20ch) + box/ctr(5ch) W-winograd pred convs + bias (no relu)."""
    psum_pool, ot_pool, ya_pool = pools
    bands = [(0, 32), (32, 32)] if H == 64 else [(0, H)]
    for bi, (r0, R) in enumerate(bands):
        n = R * T * 4
        c0 = pix_base + r0 * T * 4
        for hi, (pw, nout, xt, pb, olo) in enumerate(
                ((pwct, 20, xtc, pbc_t, 0), (pwbt, 5, xtb, pbb_t, 20))):
            ya = _wino_unit(nc, psum_pool, ot_pool, ya_pool,
                            lambda c, j, k: pw[:, c, j, k, :],
                            xt, r0, R, T, 0, nout, f"{tag}p{bi}{hi}")
            st = stage_pool.tile([32, R, T, 4], F32, tag="st",
                                 name=f"st_{tag}{bi}{hi}")
            stv = st.rearrange("p r t f -> p r f t")
            nc.scalar.activation(stv[0:nout], ya[0:nout], AF.Identity,
                                 bias=pb[0:nout])
            nc.sync.dma_start(
                out_d[olo:olo + nout, c0:c0 + n],
                st[0:nout].rearrange("p r t f -> p (r t f)"))


# ------------------------------------------------------- p5 direct path
def _conv_layer(nc, psum_pool, wt, src, dst, bias_ap, H, W, R, tag):
    n_tiles = H // R
    for o in range(NCH):
        pss = [psum_pool.tile([P, R, W], F32, tag="ps",
                              name=f"ps_{tag}_{o}_{it}")
               for it in range(n_tiles)]
        k = 0
        for c in range(NCH):
            for ky in range(3):
                for kx in range(3):
                    lhsT = wt[:, c, o, ky * 3 + kx, :]
                    for it in range(n_tiles):
                        r0 = it * R
                        rhs = src[:, c, r0 + ky:r0 + ky + R, kx:kx + W]
                        nc.tensor.matmul(pss[it][:], lhsT, rhs,
                                         start=(k == 0), stop=(k == 17))
                    k += 1
        for it in range(n_tiles):
            r0 = it * R
            nc.scalar.activation(dst[:, o, r0 + 1:r0 + 1 + R, 1:W + 1],
                                 pss[it][:], AF.Relu, bias=bias_ap[:, o])


def _preds5_head(nc, psum_pool, stage_pool, pw, pb_t, tower, out_d,
                 H, W, R, pix_base, olo, nout, tag, it):
    """One R-row band of ONE direct prediction head (cls or box/ctr)."""
    r0 = it * R
    ps = psum_pool.tile([P, R, W], F32, tag="ps", name=f"ph_{tag}_{it}")
    k = 0
    for c in range(NCH):
        for ky in range(3):
            for kx in range(3):
                t = ky * 3 + kx
                nc.tensor.matmul(ps[0:nout], pw[:, c, t, :],
                                 tower[:, c, r0 + ky:r0 + ky + R, kx:kx + W],
                                 start=(k == 0), stop=(k == 17))
                k += 1
    st = stage_pool.tile([32, R, W, 1], F32, tag="st", name=f"sh_{tag}_{it}")
    sf = st.rearrange("p r w o -> p (r w o)")
    nc.scalar.activation(sf[0:nout], ps[0:nout].rearrange("p r w -> p (r w)"),
                         AF.Identity, bias=pb_t[0:nout])
    c0 = pix_base + r0 * W
    nc.sync.dma_start(out_d[olo:olo + nout, c0:c0 + R * W], sf[0:nout])


def _preds5(nc, psum_pool, stage_pool, pwc, pwb, pbc_t, pbb_t,
            cls_tower, box_tower, out_d, H, W, R, pix_base, tag):
    n_tiles = H // R
    for it in range(n_tiles):
        r0 = it * R
        ps1 = psum_pool.tile([P, R, W], F32, tag="ps", name=f"pc_{tag}_{it}")
        ps2 = psum_pool.tile([P, R, W], F32, tag="ps", name=f"pb_{tag}_{it}")
        k = 0
        for c in range(NCH):
            for ky in range(3):
                for kx in range(3):
                    t = ky * 3 + kx
                    rc = cls_tower[:, c, r0 + ky:r0 + ky + R, kx:kx + W]
                    rb = box_tower[:, c, r0 + ky:r0 + ky + R, kx:kx + W]
                    nc.tensor.matmul(ps1[0:20], pwc[:, c, t, :], rc,
                                     start=(k == 0), stop=(k == 17))
                    nc.tensor.matmul(ps2[0:5], pwb[:, c, t, :], rb,
                                     start=(k == 0), stop=(k == 17))
                    k += 1
        st = stage_pool.tile([32, R, W, 1], F32, tag="st", name=f"s5_{tag}_{it}")
        sf = st.rearrange("p r w o -> p (r w o)")
        nc.scalar.activation(sf[0:20], ps1[0:20].rearrange("p r w -> p (r w)"),
                             AF.Identity, bias=pbc_t[0:20])
        st2 = stage_pool.tile([32, R, W, 1], F32, tag="st", name=f"s6_{tag}_{it}")
        sf2 = st2.rearrange("p r w o -> p (r w o)")
        nc.scalar.activation(sf2[0:5], ps2[0:5].rearrange("p r w -> p (r w)"),
                             AF.Identity, bias=pbb_t[0:5])
        c0 = pix_base + r0 * W
        nc.sync.dma_start(out_d[0:20, c0:c0 + R * W], sf[0:20])
        nc.sync.dma_start(out_d[20:25, c0:c0 + R * W], sf2[0:5])


# ------------------------------------------------------------ weight DMA
def _load_ww(nc, wtw_pool, sww_d, s, l, tag, fine=False):
    wt = wtw_pool.tile([P, NCH, NCH, 6, 3, P], F16, tag="ww",
                       name=f"ww_{tag}_{s}_{l}")
    if fine:
        # per-(o, j, c) splits so the first matmuls' deps clear quickly
        for o in range(NCH):
            for j in range(6):
                for c in range(NCH):
                    nc.sync.dma_start(wt[:, c, o, j], sww_d[s, l, :, c, o, j])
    else:
        for c in range(NCH):
            for o in range(NCH):
                nc.sync.dma_start(wt[:, c, o], sww_d[s, l, :, c, o])
    return wt


def _load_wp(nc, wtp_pool, swp_d, s, l, tag):
    wt = wtp_pool.tile([P, NCH, NCH, 9, P], F16, tag="wp",
                       name=f"wp_{tag}_{s}_{l}")
    for c in range(NCH):
        for o in range(NCH):
            nc.sync.dma_start(wt[:, c, o], swp_d[s, l, :, c, o])
    return wt


# ------------------------------------------------------------------ build
def _build():
    nc = bacc.Bacc("TRN2", target_bir_lowering=False, debug=False,
                   num_devices=8)

    # p3 Xt0 (host-transformed), shipped once per chain buffer
    xt3_d = nc.dram_tensor("xt3", (NCH, P, 6, 66, 16), F16,
                           kind="ExternalInput")
    xt4_d = nc.dram_tensor("xt4", (NCH, P, 6, 34, 8), F16,
                           kind="ExternalInput")
    x5_d = nc.dram_tensor("x5", (NCH, P, 18, 18), F16, kind="ExternalInput")
    sww_d = nc.dram_tensor("sww", (2, NL, P, NCH, NCH, 6, 3, P), F16,
                           kind="ExternalInput")
    swp_d = nc.dram_tensor("swp", (2, NL, P, NCH, NCH, 9, P), F16,
                           kind="ExternalInput")
    sb_d = nc.dram_tensor("sb", (2, NL, NCH, P, 1), F32, kind="ExternalInput")
    pwct_d = nc.dram_tensor("pwct", (P, NCH, 6, 3, 20), F16,
                            kind="ExternalInput")
    pwbt_d = nc.dram_tensor("pwbt", (P, NCH, 6, 3, 5), F16,
                            kind="ExternalInput")
    pwc5_d = nc.dram_tensor("pwc5", (P, NCH, 9, 20), F16,
                            kind="ExternalInput")
    pwb5_d = nc.dram_tensor("pwb5", (P, NCH, 9, 5), F16,
                            kind="ExternalInput")
    pbc_d = nc.dram_tensor("pbc", (20, 1), F32, kind="ExternalInput")
    pbb_d = nc.dram_tensor("pbb", (5, 1), F32, kind="ExternalInput")
    out_d = nc.dram_tensor("out", (25, NPIX_TOTAL), F32, kind="ExternalOutput")

    NXT3 = NCH * 6 * 66 * 16      # 12672
    NY3 = NCH * 66 * 68           # 8976
    NXT4 = NCH * 6 * 34 * 8       # 3264
    NY4 = NCH * 34 * 36           # 2448
    N5 = NCH * 18 * 18            # 648

    with tile.TileContext(nc) as tc:
        with (
            tc.tile_pool(name="resident", bufs=1) as res_pool,
            tc.tile_pool(name="wtw", bufs=2) as wtw_pool,
            tc.tile_pool(name="wtp", bufs=1) as wtp_pool,
            tc.tile_pool(name="psum", bufs=8, space="PSUM") as psum_pool,
            tc.tile_pool(name="stage", bufs=2) as stage_pool,
            tc.tile_pool(name="it", bufs=5) as it_pool,
            tc.tile_pool(name="ot", bufs=18) as ot_pool,
            tc.tile_pool(name="ya", bufs=4) as ya_pool,
        ):
            xtc_f = res_pool.tile([P, NXT3], F16, name="xtc")
            xtb_f = res_pool.tile([P, NXT3], F16, name="xtb")
            yc_f = res_pool.tile([P, NY3], F16, name="yc")
            yb_f = res_pool.tile([P, NY3], F16, name="yb")

            sbias = res_pool.tile([P, 2, NL, NCH, 1], F32, name="sbias")
            pwct = res_pool.tile([P, NCH, 6, 3, 20], F16, name="pwct")
            pwbt = res_pool.tile([P, NCH, 6, 3, 5], F16, name="pwbt")
            pwc5 = res_pool.tile([P, NCH, 9, 20], F16, name="pwc5")
            pwb5 = res_pool.tile([P, NCH, 9, 5], F16, name="pwb5")
            pbc_t = res_pool.tile([32, 1], F32, name="pbct")
            pbb_t = res_pool.tile([32, 1], F32, name="pbbt")

            xtc = _xt_view(xtc_f, 0, 64, 16)
            xtb = _xt_view(xtb_f, 0, 64, 16)
            yc = _y_view(yc_f, 0, 64, 16)
            yb = _y_view(yb_f, 0, 64, 16)
            # pass-b carvings
            xtc4 = _xt_view(yc_f, 0, 32, 8)
            yc4 = _y_view(yc_f, NXT4, 32, 8)
            xtb4 = _xt_view(yb_f, 0, 32, 8)
            yb4 = _y_view(yb_f, NXT4, 32, 8)
            # p5 runs inside pass A (its direct matmuls fill tensor idle
            # and cost no vector-engine work) -> static buffers
            x5_f = res_pool.tile([P, 4 * N5], F16, name="x5f")
            v5 = [_pad_view(x5_f, i * N5, 16, 16) for i in range(4)]

            pools = (psum_pool, ot_pool, ya_pool)

            # ---- startup DMAs: first-layer weights + cls Xt0, then rest
            # the very first matmuls' Xt chunks lead the sync queue
            for c in range(NCH):
                nc.sync.dma_start(xtc[:, c, 0, 0:33], xt3_d[c, :, 0, 0:33])
            ww00 = _load_ww(nc, wtw_pool, sww_d, 0, 0, "a", fine=True)
            # remaining Xt0 chunks ride the gpsimd SWDGE queue, parallel to
            # the startup-critical weight DMAs on the sync queue
            for j in range(6):
                for c in range(NCH):
                    if j > 0:
                        nc.gpsimd.dma_start(xtc[:, c, j, 0:33],
                                            xt3_d[c, :, j, 0:33])
                    nc.gpsimd.dma_start(xtc[:, c, j, 33:66],
                                        xt3_d[c, :, j, 33:66])
            for c in range(NCH):
                nc.sync.dma_start(xtb[:, c], xt3_d[c])
            nc.gpsimd.dma_start(
                sbias[:],
                sb_d[:].rearrange("s l a p o -> p (s l a o)")
                       .rearrange("p (s l a o) -> p s l a o",
                                  s=2, l=NL, a=NCH))
            nc.gpsimd.dma_start(pwct[:], pwct_d[:])
            nc.gpsimd.dma_start(pwbt[:], pwbt_d[:])
            nc.gpsimd.dma_start(pwc5[:], pwc5_d[:])
            nc.gpsimd.dma_start(pwb5[:], pwb5_d[:])
            nc.gpsimd.dma_start(pbc_t[0:20], pbc_d[:])
            nc.gpsimd.dma_start(pbb_t[0:5], pbb_d[:])
            for c in range(NCH):
                nc.gpsimd.dma_start(v5[0][:, c], x5_d[c])
            _zero_y_ring(nc, yc, 64, 16)
            _zero_y_ring(nc, yb, 64, 16)
            for i in range(1, 4):
                _zero_ring(nc, v5[i], 16, 16)

            # ---- pass A: p3 winograd + p5 direct, chains interleaved
            # p5 rotation: cls v0->v1->v3->v1->v3 (tower v3);
            #              box v0->v2->v0->v2->v0 (tower v0)
            cls_chain = [(0, 1), (1, 3), (3, 1), (1, 3)]
            box_chain = [(0, 2), (2, 0), (0, 2), (2, 0)]
            ww = {(0, 0): ww00}

            def get_ww(s, l, tag):
                if (s, l) not in ww:
                    ww[(s, l)] = _load_ww(nc, wtw_pool, sww_d, s, l, tag)
                return ww[(s, l)]

            ycp = _pad_view(yc_f, 0, 64, 64)
            ybp = _pad_view(yb_f, 0, 64, 64)
            y4cp = _pad_view(yc_f, NXT4, 32, 32)
            y4bp = _pad_view(yb_f, NXT4, 32, 32)

            for l in range(NL):
                for s, xt, y, chain in ((0, xtc, yc, cls_chain),
                                        (1, xtb, yb, box_chain)):
                    wt = get_ww(s, l, "a")
                    wp = _load_wp(nc, wtp_pool, swp_d, s, l, "a")
                    if l > 0:
                        _intrans(nc, it_pool, y, xt, 64, 16, f"a{s}{l}")
                    if l == NL - 1:
                        # towers in plain layout: preds run as direct convs
                        yp = ycp if s == 0 else ybp
                        _zero_ring(nc, yp, 64, 64)
                        _wino_layer(nc, pools, wt, xt, yp, sbias[:, s, l],
                                    64, 16, f"a{s}{l}", plain=True)
                    else:
                        _wino_layer(nc, pools, wt, xt, y, sbias[:, s, l],
                                    64, 16, f"a{s}{l}")
                    si, di = chain[l]
                    _conv_layer(nc, psum_pool, wp, v5[si], v5[di],
                                sbias[:, s, l], 16, 16, 16, f"a5{s}{l}")
                    if l == NL - 1 and s == 0:
                        # cls-head pred bands run during the box chain's
                        # final layer (its transforms own DVE/gpsimd then)
                        for it in range(8):
                            _preds5_head(nc, psum_pool, stage_pool, pwc5,
                                         pbc_t, ycp, out_d, 64, 64, 8, 0,
                                         0, 20, "a3c", it)
                    # prefetch next (s, l) winograd weights
                    nl_s, nl_l = (1, l) if s == 0 else (0, l + 1)
                    if nl_l < NL:
                        get_ww(nl_s, nl_l, "a")
            for it in range(8):
                _preds5_head(nc, psum_pool, stage_pool, pwb5, pbb_t, ybp,
                             out_d, 64, 64, 8, 0, 20, 5, "a3b", it)
            _preds5(nc, psum_pool, stage_pool, pwc5, pwb5, pbc_t, pbb_t,
                    v5[3], v5[0], out_d, 16, 16, 16, 5120, "a5")
            # p4 staging into regions freed once the p3 preds are done
            for c in range(NCH):
                nc.sync.dma_start(xtc4[:, c], xt4_d[c])
                nc.sync.dma_start(xtb4[:, c], xt4_d[c])
            _zero_y_ring(nc, yc4, 32, 8)
            _zero_y_ring(nc, yb4, 32, 8)

            # ---- pass B: p4 winograd stems + direct preds
            for l in range(NL):
                for s, xt4, y4 in ((0, xtc4, yc4), (1, xtb4, yb4)):
                    # reload winograd weights for pass b (pool rotation)
                    wt = _load_ww(nc, wtw_pool, sww_d, s, l, "b")
                    if l > 0:
                        _intrans(nc, it_pool, y4, xt4, 32, 8, f"b{s}{l}")
                    if l == NL - 1:
                        yp = y4cp if s == 0 else y4bp
                        _zero_ring(nc, yp, 32, 32)
                        _wino_layer(nc, pools, wt, xt4, yp, sbias[:, s, l],
                                    32, 8, f"b{s}{l}", plain=True)
                    else:
                        _wino_layer(nc, pools, wt, xt4, y4, sbias[:, s, l],
                                    32, 8, f"b{s}{l}")
            _preds5(nc, psum_pool, stage_pool, pwc5, pwb5, pbc_t, pbb_t,
                    y4cp, y4bp, out_d, 32, 32, 16, 4096, "b3")

    nc.compile()
    return nc


# ------------------------------------------------------------- host side
def _wino_xt(x):
    """x [256, H, W] fp32 -> Xt [2, 128, 6, H+2, W//4] fp16."""
    Cc, H, W = x.shape
    T = W // 4
    xp = np.pad(x, ((0, 0), (1, 1), (1, 3))).astype(np.float32)
    xa = [xp[:, :, a:a + 4 * T:4] for a in range(6)]
    g = xa[3] - xa[1]
    h = xa[4] - xa[2]
    r3 = g * 2 + h
    r4 = g * -2 + h
    r5 = (xa[5] - xa[3]) + g * -4
    r0 = (xa[0] - xa[2]) * 4 + h
    r2 = (xa[1] - xa[2]) * 4 + (xa[4] - xa[3])
    r1 = (xa[1] + xa[2]) * -4 + (xa[3] + xa[4])
    xt = np.stack([r0, r1, r2, r3, r4, r5])          # [6, 256, H+2, T]
    xt = xt.reshape(6, NCH, P, H + 2, T).transpose(1, 2, 0, 3, 4)
    return np.ascontiguousarray(xt, dtype=np.float16)


def _pack_ww(wcls, wbox):
    """[2][NL, 256, 256, 3, 3] -> [2, NL, 128ip, 2ic, 2oc, 6j, 3k, 128op]."""
    out = np.empty((2, NL, P, NCH, NCH, 6, 3, P), np.float16)
    for s, w in enumerate((wcls, wbox)):
        for l in range(NL):
            t = np.einsum('ja,oika->oikj', G_MAT, w[l].astype(np.float64))
            t = t.reshape(NCH, P, NCH, P, 3, 6).transpose(3, 2, 0, 5, 4, 1)
            out[s, l] = t.astype(np.float16)
    return np.ascontiguousarray(out)


def _pack_pred_wt(w):
    """[n, 256, 3, 3] -> [128ip, 2ic, 6j, 3k, n]."""
    n = w.shape[0]
    t = np.einsum('ja,oika->oikj', G_MAT, w.astype(np.float64))
    t = t.reshape(n, NCH, P, 3, 6).transpose(2, 1, 4, 3, 0)
    return np.ascontiguousarray(t, dtype=np.float16)


def _pack_stem_wp(wcls, wbox):
    w = np.stack([wcls, wbox]).reshape(2, NL, NCH, P, NCH, P, 3, 3)
    w = w.transpose(0, 1, 5, 4, 2, 6, 7, 3)
    return np.ascontiguousarray(w.reshape(2, NL, P, NCH, NCH, 9, P),
                                dtype=np.float16)


def _pack_pred_wp(w):
    n = w.shape[0]
    w = w.reshape(n, NCH, P, 3, 3).transpose(2, 1, 3, 4, 0)
    return np.ascontiguousarray(w.reshape(P, NCH, 9, n), dtype=np.float16)


def kernel(p3, p4, p5, stem_cls_w, stem_cls_b, stem_box_w, stem_box_b,
           pred_cls_w, pred_cls_b, pred_box_w, pred_box_b,
           pred_ctr_w, pred_ctr_b):
    if 'nc' not in _cached:
        _cached['nc'] = _build()
    nc = _cached['nc']

    B = p3.shape[0]
    scw = np.asarray(stem_cls_w, np.float32)
    sbw = np.asarray(stem_box_w, np.float32)
    shared = {
        "sww": _pack_ww(scw, sbw),
        "swp": _pack_stem_wp(scw, sbw),
        "sb": np.ascontiguousarray(
            np.stack([stem_cls_b, stem_box_b]).reshape(2, NL, NCH, P, 1),
            dtype=np.float32),
        "pwct": _pack_pred_wt(np.asarray(pred_cls_w)),
        "pwbt": _pack_pred_wt(
            np.concatenate([pred_box_w, pred_ctr_w], axis=0)),
        "pwc5": _pack_pred_wp(np.asarray(pred_cls_w)),
        "pwb5": _pack_pred_wp(
            np.concatenate([pred_box_w, pred_ctr_w], axis=0)),
        "pbc": np.asarray(pred_cls_b, np.float32).reshape(20, 1),
        "pbb": np.concatenate(
            [pred_box_b, pred_ctr_b]).astype(np.float32).reshape(5, 1),
    }
    in_maps = []
    for b in range(B):
        m = dict(shared)
        m["xt3"] = _wino_xt(np.asarray(p3[b], np.float32))
        m["xt4"] = _wino_xt(np.asarray(p4[b], np.float32))
        m["x5"] = np.pad(
            np.asarray(p5[b], np.float16).reshape(NCH, P, 16, 16),
            ((0, 0), (0, 0), (1, 1), (1, 1)))
        in_maps.append(m)

    res = run_bass_kernel_spmd(nc, in_maps, core_ids=list(range(B)),
                               **_run_opts)
    _last['res'] = res
    out = np.stack([r["out"].T for r in res.results])
    return np.ascontiguousarray(out, dtype=np.float32)
